# revision 20
# baseline (speedup 1.0000x reference)
"""Trainium2 Bass kernel for nn_AttentionBlock (GroupNorm + QKV + MHA).

Sharding: tensor-parallel over the H=8 heads, one head per NeuronCore.
Each core:
  - loads x^T (C on partitions) once, in bf16 (halves DMA + stats time),
  - computes GroupNorm statistics with bn_stats + tiny indicator matmuls,
  - folds the (data-dependent) GroupNorm affine INTO the QKV weights:
      xn = x*A + B  =>  qkv = x @ (A*W) + (b + B@W)
  - computes Q^T/K^T packed as [Q;K]; the swapped [K;Q] layout needed for
    concurrent 64x128 row-tile QK^T matmuls (tile_position) is produced by
    two SBUF->SBUF DMA copies per chunk instead of a second projection,
  - softmax without max-subtraction (scores are O(1) here), denominator via
    a ones-column appended to V in the P^T-orientation AV matmul,
  - epilogue transposes the [ch, t] PSUM output to token-major via the PE,
    so the softmax denominator lands spread over 128 partitions and the
    reciprocal + normalization are cheap per-partition DVE ops.
Host side only reorders/permutes arrays and concatenates per-head outputs.
"""
import sys

try:
    import concourse.bass as bass  # noqa: F401
except ImportError:
    sys.path.insert(0, "/opt/trn_rl_repo")
    import concourse.bass as bass  # noqa: F401

import numpy as np
import ml_dtypes
import concourse.tile as tile
from concourse import bacc, mybir
from concourse.bass_utils import run_bass_kernel_spmd

F32 = mybir.dt.float32
F32R = mybir.dt.float32r
F16 = mybir.dt.float16
BF16 = mybir.dt.bfloat16
AF = mybir.ActivationFunctionType
OP = mybir.AluOpType

T = 4096          # tokens (64*64)
C = 512           # channels
H = 8             # heads
CH = C // H       # 64 head dim
G = 32            # groupnorm groups
EPS = 1e-6
QKS = float(CH) ** -0.25   # scale applied to q and k each
NCORES = 8
TC = 512          # t-chunk width
NTC = T // TC     # 8 chunks
NSB = T // 128    # 32 s-blocks
NWARM = 28        # PE warmup matmuls
DEBUG_DUMP = False # add debug DRAM dumps of intermediates


def build_nc():
    nc = bacc.Bacc()
    xt = nc.declare_dram_parameter("xt", [C, T], BF16, isOutput=False)
    wqk = nc.declare_dram_parameter("wqk", [C, 128], BF16, isOutput=False)
    wv = nc.declare_dram_parameter("wv", [C, CH], BF16, isOutput=False)
    bqk = nc.declare_dram_parameter("bqk", [128, 1], F32, isOutput=False)
    bvT = nc.declare_dram_parameter("bvT", [CH, 1], F32, isOutput=False)
    gns = nc.declare_dram_parameter("gns", [128, 4], F32, isOutput=False)
    gnb = nc.declare_dram_parameter("gnb", [128, 4], F32, isOutput=False)
    ind = nc.declare_dram_parameter("ind", [128, 8], F32R, isOutput=False)
    indt = nc.declare_dram_parameter("indt", [8, 128], F32R, isOutput=False)
    outT = nc.declare_dram_parameter("outT", [T, CH], F32, isOutput=True)
    dbg = None
    if DEBUG_DUMP:
        dbg = {
            "dbg_vt": nc.declare_dram_parameter("dbg_vt", [128, NSB * 80], F16, isOutput=True),
            "dbg_qk1": nc.declare_dram_parameter("dbg_qk1", [128, T], F16, isOutput=True),
            "dbg_kq2": nc.declare_dram_parameter("dbg_kq2", [128, T], F16, isOutput=True),
        }

    with tile.TileContext(nc) as tc:
        _build_body(nc, tc, xt, wqk, wv, bqk, bvT, gns, gnb, ind, indt, outT,
                    dbg)
    nc.finalize()
    return nc


def _build_body(nc, tc, xt, wqk, wv, bqk, bvT, gns, gnb, ind, indt, outT,
                dbg=None):
    from contextlib import ExitStack
    ctx = ExitStack()
    with ctx:
        const = ctx.enter_context(tc.tile_pool(name="const", bufs=1))
        big = ctx.enter_context(tc.tile_pool(name="big", bufs=1))
        work = ctx.enter_context(tc.tile_pool(name="work", bufs=2))
        epi = ctx.enter_context(tc.tile_pool(name="epi", bufs=2))
        ppool = ctx.enter_context(tc.tile_pool(name="ppool", bufs=4))
        ps_pair = ctx.enter_context(tc.tile_pool(name="ps_pair", bufs=2, space="PSUM"))
        ps_o = ctx.enter_context(tc.tile_pool(name="ps_o", bufs=2, space="PSUM"))
        ps_sm = ps_o

        # ---------------- loads ----------------
        # weights first (small) so the PE warmup can start immediately
        wqk_raw, wv_raw = [], []
        for k in range(4):
            a = const.tile([128, 128], BF16, tag=f"wqkr{k}")
            nc.sync.dma_start(out=a, in_=wqk[128 * k:128 * (k + 1), :])
            wqk_raw.append(a)
            v = const.tile([128, CH], BF16, tag=f"wvr{k}")
            nc.sync.dma_start(out=v, in_=wv[128 * k:128 * (k + 1), :])
            wv_raw.append(v)
        # whole-tile x loads (8KB contiguous rows aggregate best); arrival
        # order 0,2,1,3 so the DVE (tiles 0,1,3) and ACT (tile 2) stats
        # pipelines both start as early as possible.
        xtk = [None] * 4
        for k in (0, 2, 1, 3):
            t = big.tile([128, T], BF16, tag=f"xt{k}")
            nc.sync.dma_start(out=t, in_=xt[128 * k:128 * (k + 1), :])
            xtk[k] = t
        bqk_t = const.tile([128, 1], F32, tag="bqk")
        nc.sync.dma_start(out=bqk_t, in_=bqk[:, :])
        bvT_t = const.tile([CH, 1], F32, tag="bvT")
        nc.sync.dma_start(out=bvT_t, in_=bvT[:, :])
        gns_t = const.tile([128, 4], F32, tag="gns")
        nc.sync.dma_start(out=gns_t, in_=gns[:, :])
        gnb_t = const.tile([128, 4], F32, tag="gnb")
        nc.sync.dma_start(out=gnb_t, in_=gnb[:, :])
        ind_t = const.tile([128, 8], F32R, tag="ind")
        nc.sync.dma_start(out=ind_t, in_=ind[:, :])
        indt_t = const.tile([8, 128], F32R, tag="indt")
        nc.sync.dma_start(out=indt_t, in_=indt[:, :])

        # f16 identity for PE transposes (V^T -> vt, epilogue -> token-major)
        ident = const.tile([128, 128], F16, tag="ident")
        from concourse.masks import make_identity
        make_identity(nc, ident)

        # PE warmup: dummy matmuls paced by the arriving xt quarters keep
        # the HAM activity monitor warm through the DMA+stats startup so
        # phase B starts at 2.4 GHz instead of 1.2 GHz. Results unused.
        warm_ps = ps_sm.tile([128, TC], F32, name="warm_ps", tag="psO")
        for w_i in range(NWARM):
            k = w_i % 4
            q = (w_i // 4) % 4
            nc.tensor.matmul(out=warm_ps, lhsT=wqk_raw[k],
                             rhs=xtk[k][:, bass.ts(q, 1024)][:, 0:512],
                             start=True, stop=True)

        # ---------------- phase A: groupnorm stats -> folded weights ------
        # DVE bn_stats handles k-tiles {0,1,3}; the (otherwise idle) ACT
        # engine handles tile 2 via accum_out running sums of x and x^2.
        RS = work.tile([128, 8], F32R, tag="RS")
        tmp1 = work.tile([128, 1], F32, tag="tmp1")
        scr = big.tile([128, T], BF16, tag="scr")
        sx = work.tile([128, 1], F32, tag="sx")
        sx2 = work.tile([128, 1], F32, tag="sx2")
        for k in range(4):
            if k == 2:
                nc.scalar.activation(out=scr, in_=xtk[k], func=AF.Identity,
                                     accum_out=sx)
                nc.scalar.activation(out=scr, in_=xtk[k], func=AF.Square,
                                     accum_out=sx2)
                nc.vector.tensor_scalar_mul(out=RS[:, 2 * k:2 * k + 1],
                                            in0=sx, scalar1=1.0 / T)
                nc.vector.tensor_scalar_mul(out=RS[:, 2 * k + 1:2 * k + 2],
                                            in0=sx2, scalar1=1.0 / T)
                continue
            st = work.tile([128, 8, 6], F32, tag="st")
            for sub in range(8):
                nc.vector.bn_stats(out=st[:, sub, :],
                                   in_=xtk[k][:, 512 * sub:512 * (sub + 1)])
            mv = work.tile([128, 2], F32, tag="mv")
            nc.vector.bn_aggr(out=mv, in_=st)
            # RS[:, 2k] = mean ; RS[:, 2k+1] = var + mean^2 = E[x^2]
            nc.vector.tensor_copy(out=RS[:, 2 * k:2 * k + 1], in_=mv[:, 0:1])
            nc.vector.tensor_mul(out=tmp1, in0=mv[:, 0:1], in1=mv[:, 0:1])
            nc.vector.tensor_add(out=RS[:, 2 * k + 1:2 * k + 2], in0=tmp1,
                                 in1=mv[:, 1:2])
        psG = ps_sm.tile([128, 8], F32, tag="psO")
        nc.tensor.matmul(out=psG[0:8, :], lhsT=ind_t, rhs=RS, start=True,
                         stop=True)
        # group stats (8 groups-per-tile x 4 tiles)
        gm = work.tile([8, 4], F32, tag="gm")
        nc.vector.tensor_scalar_mul(out=gm, in0=psG[0:8, 0:8:2], scalar1=1.0 / 16)
        ge2 = work.tile([8, 4], F32, tag="ge2")
        nc.vector.tensor_scalar_mul(out=ge2, in0=psG[0:8, 1:8:2], scalar1=1.0 / 16)
        gv = work.tile([8, 4], F32, tag="gv")
        t8 = work.tile([8, 4], F32, tag="t8")
        nc.vector.tensor_mul(out=t8, in0=gm, in1=gm)
        nc.vector.tensor_sub(out=gv, in0=ge2, in1=t8)
        # rstd = rsqrt(var+eps), one Newton step for accuracy
        gve = work.tile([8, 4], F32, tag="gve")
        nc.vector.tensor_scalar_add(out=gve, in0=gv, scalar1=EPS)
        sq0 = work.tile([8, 4], F32, tag="sq0")
        nc.scalar.activation(out=sq0, in_=gve, func=AF.Sqrt)
        r0 = work.tile([8, 4], F32, tag="r0")
        nc.vector.reciprocal(out=r0, in_=sq0)
        nc.vector.tensor_mul(out=t8, in0=r0, in1=r0)
        nc.vector.tensor_mul(out=t8, in0=t8, in1=gve)
        nc.vector.tensor_scalar(out=t8, in0=t8, scalar1=-0.5, scalar2=1.5,
                                op0=OP.mult, op1=OP.add)
        grstd = work.tile([8, 4], F32, tag="grstd")
        nc.vector.tensor_mul(out=grstd, in0=r0, in1=t8)
        # G[:, 2k] = gmean_k ; G[:, 2k+1] = grstd_k
        Gt = work.tile([8, 8], F32R, tag="Gt")
        nc.vector.tensor_copy(out=Gt[:, 0:8:2], in_=gm)
        nc.vector.tensor_copy(out=Gt[:, 1:8:2], in_=grstd)
        psB = ps_sm.tile([128, 8], F32, tag="psO")
        nc.tensor.matmul(out=psB, lhsT=indt_t, rhs=Gt, start=True, stop=True)
        # per-channel A = rstd*gn_scale ; B = gn_bias - mean*A
        Av = work.tile([128, 4], F32, tag="Av")
        nc.vector.tensor_mul(out=Av, in0=psB[:, 1:8:2], in1=gns_t)
        t128 = work.tile([128, 4], F32, tag="t128")
        nc.vector.tensor_mul(out=t128, in0=psB[:, 0:8:2], in1=Av)
        Bc = work.tile([128, 4], F32, tag="Bc")
        nc.vector.tensor_sub(out=Bc, in0=gnb_t, in1=t128)
        Br = work.tile([128, 4], F32, tag="Br")
        nc.vector.tensor_copy(out=Br, in_=Bc)
        Bsr = work.tile([128, 4], F32, tag="Bsr")
        nc.vector.tensor_scalar_mul(out=Bsr, in0=Bc, scalar1=QKS)
        # column-duplicated copies: fp32r matmuls need an even moving dim
        Bsr2 = work.tile([128, 8], BF16, tag="Bsr2")
        nc.vector.tensor_copy(out=Bsr2[:, 0:8:2], in_=Bsr)
        nc.vector.tensor_copy(out=Bsr2[:, 1:8:2], in_=Bsr)
        Br2 = work.tile([128, 8], BF16, tag="Br2")
        nc.vector.tensor_copy(out=Br2[:, 0:8:2], in_=Br)
        nc.vector.tensor_copy(out=Br2[:, 1:8:2], in_=Br)
        Aqk = work.tile([128, 4], F32, tag="Aqk")
        nc.vector.tensor_scalar_mul(out=Aqk, in0=Av, scalar1=QKS)
        # scaled weights, cast to bf16 to match the bf16 x
        wqk_s, wv_s = [], []
        for k in range(4):
            a = const.tile([128, 128], BF16, tag=f"wqks{k}")
            nc.vector.tensor_scalar_mul(out=a, in0=wqk_raw[k],
                                        scalar1=Aqk[:, k:k + 1])
            wqk_s.append(a)
            v = const.tile([128, CH], BF16, tag=f"wvs{k}")
            nc.vector.tensor_scalar_mul(out=v, in0=wv_raw[k],
                                        scalar1=Av[:, k:k + 1])
            wv_s.append(v)
        # folded biases: b'' = s*b_host + (s*B) @ W_raw   (qk)  /  b + B @ Wv
        psQKb = ps_sm.tile([128, 2], F32, tag="psO")
        for k in range(4):
            nc.tensor.matmul(out=psQKb, lhsT=wqk_raw[k],
                             rhs=Bsr2[:, 2 * k:2 * k + 2],
                             start=(k == 0), stop=(k == 3))
        bqk_dev = const.tile([128, 1], F32, tag="bqk_dev")
        nc.vector.tensor_add(out=bqk_dev, in0=psQKb[:, 0:1], in1=bqk_t)
        psBvT = ps_sm.tile([CH, 2], F32, tag="psO")
        for k in range(4):
            nc.tensor.matmul(out=psBvT, lhsT=wv_raw[k],
                             rhs=Br2[:, 2 * k:2 * k + 2],
                             start=(k == 0), stop=(k == 3))
        bv_dev = const.tile([CH, 1], F32, tag="bv_dev")
        nc.vector.tensor_add(out=bv_dev, in0=psBvT[:, 0:1], in1=bvT_t)

        # ---------------- phase B: projections ----------------
        qk1 = big.tile([128, T], F16, tag="qk1")   # [Q; K]
        kq2 = big.tile([128, T], F16, tag="kq2")   # [K; Q] via DMA copies

        for cc in range(NTC):
            sl = bass.ts(cc, TC)
            ps1 = ps_pair.tile([128, 1536], F32, tag="pp")
            for k in range(4):
                nc.tensor.matmul(out=ps1[:, 0:512], lhsT=wqk_s[k],
                                 rhs=xtk[k][:, sl], start=(k == 0),
                                 stop=(k == 3))
            # bias-add on ACT (idle during this phase), writes f16
            nc.scalar.activation(out=qk1[:, sl], in_=ps1[:, 0:512],
                                 func=AF.Identity, bias=bqk_dev, scale=1.0)
            # swapped layout via SBUF->SBUF DMA (partition-crossing copy)
            nc.sync.dma_start(out=kq2[0:64, sl], in_=qk1[64:128, sl])
            nc.sync.dma_start(out=kq2[64:128, sl], in_=qk1[0:64, sl])
        # V^T projection, then DMA-transpose (XBAR) into token-major vt
        # (with a ones column for the softmax denominator). The transposes
        # run on the idle DMA engines instead of the PE.
        vts = big.tile([CH, T], F16, tag="vts")
        # inner stride 80: XBAR-transpose output rows need 32-byte alignment
        vt = big.tile([128, NSB, 80], F16, tag="vt")
        nc.vector.memset(vt[:, :, CH:CH + 1], 1.0)
        for cc in range(NTC):
            sl = bass.ts(cc, TC)
            psVT = ps_sm.tile([CH, TC], F32, name=f"psVT_{cc}", tag="psO")
            for k in range(4):
                nc.tensor.matmul(out=psVT, lhsT=wv_s[k], rhs=xtk[k][:, sl],
                                 start=(k == 0), stop=(k == 3))
            nc.scalar.activation(out=vts[:, sl], in_=psVT, func=AF.Identity,
                                 bias=bv_dev, scale=1.0)
            # one XBAR transpose per chunk: out[p, j, c] = vts[c, j*128+p]
            nc.sync.dma_start(out=vt[:, 4 * cc:4 * cc + 4, 0:CH],
                              in_=vts[:, sl], transpose=True)

        # ---------------- phase C: attention ----------------
        # S^T tiles grouped 3 s-blocks wide so each ACT exp covers 1536
        # columns (amortizes per-instruction overhead on the bottleneck
        # engine). mm1 runs as concurrent 64x128 row-tile pairs.
        NP = NSB // 2
        NT_C = 11  # 10 tiles of 3 blocks + 1 tile of 2 blocks

        def epi_finish(state):
            # psO holds [out^T; den] (65 x 512). Transpose to token-major so
            # the denominator spreads across partitions, then the reciprocal
            # and the normalizing multiply are cheap per-partition DVE ops.
            cc0, sAB0 = state
            # inner dim padded to CH+2 so each j-slice is 4-byte aligned in PSUM
            psT = ps_o.tile([128, 4, CH + 2], F16, name=f"psT_{cc0}", tag="psO")
            for j in range(4):
                nc.tensor.transpose(out=psT[:, j, 0:CH + 1],
                                    in_=sAB0[:, bass.ts(j, 128)],
                                    identity=ident[0:CH + 1, 0:CH + 1])
            rden = work.tile([128, 4], F32, name=f"rden_{cc0}", tag="rden")
            nc.vector.reciprocal(out=rden, in_=psT[:, :, CH])
            osb = work.tile([128, 4, CH], F32, name=f"osb_{cc0}", tag="osb")
            for j in range(4):
                nc.vector.tensor_scalar_mul(out=osb[:, j, :],
                                            in0=psT[:, j, 0:CH],
                                            scalar1=rden[:, j:j + 1])
                nc.sync.dma_start(
                    out=outT[TC * cc0 + 128 * j:TC * cc0 + 128 * (j + 1), :],
                    in_=osb[:, j, :])

        def mm1_pair(cc, pr, tiles):
            sl = bass.ts(cc, TC)
            for half, blk in ((0, 2 * pr), (1, 2 * pr + 1)):
                ti, col = divmod(blk, 3)
                dst = tiles[ti][1][:, 512 * col:512 * (col + 1)]
                if half == 0:
                    nc.tensor.matmul(out=dst,
                                     lhsT=kq2[0:64, bass.ts(blk, 128)],
                                     rhs=qk1[0:64, sl], start=True, stop=True,
                                     tile_position=(0, 0))
                else:
                    nc.tensor.matmul(out=dst,
                                     lhsT=qk1[64:128, bass.ts(blk, 128)],
                                     rhs=kq2[64:128, sl], start=True,
                                     stop=True, tile_position=(64, 0))

        def exp_tile(tiles, ti):
            nblk, pp = tiles[ti]
            w = 512 * nblk
            P = ppool.tile([128, 1536], F16, tag="P")
            nc.scalar.activation(out=P[:, 0:w], in_=pp[:, 0:w], func=AF.Exp)
            return P

        def mm2_tile(psO, ti, P, nblk, first, last):
            for c in range(nblk):
                blk = 3 * ti + c
                nc.tensor.matmul(out=psO[0:CH + 1, :],
                                 lhsT=vt[:, blk, 0:CH + 1],
                                 rhs=P[:, 512 * c:512 * (c + 1)],
                                 start=(first and c == 0),
                                 stop=(last and c == nblk - 1))

        pending = None
        for cc in range(NTC):
            psO = ps_o.tile([128, TC], F32, name=f"psO_{cc}", tag="psO")
            tiles = {}
            done_upto = -1
            flushed = []   # (ti, P, nblk)
            for pr in range(NP):
                for blk in (2 * pr, 2 * pr + 1):
                    ti = blk // 3
                    if ti not in tiles:
                        nblk = 2 if ti == NT_C - 1 else 3
                        tiles[ti] = (nblk, ps_pair.tile(
                            [128, 1536], F32, name=f"pp_{cc}_{ti}", tag="pp"))
                mm1_pair(cc, pr, tiles)
                pair_done = 2 * pr + 1
                while True:
                    ti = done_upto + 1
                    if ti >= NT_C or ti not in tiles:
                        break
                    nblk = tiles[ti][0]
                    if 3 * ti + nblk - 1 > pair_done:
                        break
                    P = exp_tile(tiles, ti)
                    flushed.append((ti, P, nblk))
                    done_upto = ti
                    if len(flushed) >= 2:
                        pti, pP, pn = flushed[-2]
                        mm2_tile(psO, pti, pP, pn, first=(pti == 0),
                                 last=False)
                    if pending is not None and ti == 2:
                        epi_finish(pending)
                        pending = None
            lti, lP, ln = flushed[-1]
            mm2_tile(psO, lti, lP, ln, first=False, last=True)
            if cc == 0 and dbg is not None:
                nc.sync.dma_start(out=dbg["dbg_vt"][:, :],
                                  in_=vt[:, :, :])
                nc.sync.dma_start(out=dbg["dbg_qk1"][:, :], in_=qk1[:, :])
                nc.sync.dma_start(out=dbg["dbg_kq2"][:, :], in_=kq2[:, :])
            # pull [out^T; den] into SBUF (f16) so the PE can transpose it;
            # this also frees psO's PSUM slot early for the ring.
            sAB = epi.tile([CH + 1, TC], F16, name=f"sAB_{cc}", tag="sAB")
            nc.vector.tensor_copy(out=sAB, in_=psO[0:CH + 1, :])
            pending = (cc, sAB)
        epi_finish(pending)


_NC_CACHE = None
TRACE = False          # set True (e.g. from test.py) to capture an NTFF profile
TMPDIR = None          # set to a path to keep NEFF/NTFF artifacts
LAST_RESULT = None     # BassKernelResults of the most recent kernel() call


def _get_nc():
    global _NC_CACHE
    if _NC_CACHE is None:
        _NC_CACHE = build_nc()
    return _NC_CACHE


def kernel(x, gn_scale, gn_bias, w_qkv, b_qkv):
    x = np.asarray(x, dtype=np.float32)
    gn_scale = np.asarray(gn_scale, dtype=np.float32)
    gn_bias = np.asarray(gn_bias, dtype=np.float32)
    w_qkv = np.asarray(w_qkv, dtype=np.float32)
    b_qkv = np.asarray(b_qkv, dtype=np.float32)

    spatial = x.shape[:-1]
    xt = np.ascontiguousarray(x.reshape(T, C).T).astype(ml_dtypes.bfloat16)
    w3 = w_qkv.reshape(C, C, 3)
    wq, wk, wv = w3[..., 0], w3[..., 1], w3[..., 2]          # (C, C) each
    b3 = b_qkv.reshape(C, 3)
    bq, bk, bvv = b3[:, 0], b3[:, 1], b3[:, 2]

    # indicator matrices: channel partition p -> group j = p // 16
    p_idx = np.arange(128)
    ind = (p_idx[:, None] // 16 == np.arange(8)[None, :]).astype(np.float32)
    indt = np.ascontiguousarray(ind.T)
    gns_dev = np.ascontiguousarray(gn_scale.reshape(4, 128).T)
    gnb_dev = np.ascontiguousarray(gn_bias.reshape(4, 128).T)

    in_maps = []
    for h in range(H):
        hsl = slice(CH * h, CH * (h + 1))
        wq_h, wk_h, wv_h = wq[:, hsl], wk[:, hsl], wv[:, hsl]
        in_maps.append({
            "xt": xt,
            "wqk": np.concatenate([wq_h, wk_h], axis=1).astype(ml_dtypes.bfloat16),
            "wv": np.ascontiguousarray(wv_h).astype(ml_dtypes.bfloat16),
            "bqk": (QKS * np.concatenate([bq[hsl], bk[hsl]])).reshape(128, 1),
            "bvT": bvv[hsl].reshape(CH, 1).copy(),
            "gns": gns_dev,
            "gnb": gnb_dev,
            "ind": ind,
            "indt": indt,
        })

    nc = _get_nc()
    res = run_bass_kernel_spmd(nc, in_maps, list(range(NCORES)), trace=TRACE,
                               tmpdir=TMPDIR)
    global LAST_RESULT
    LAST_RESULT = res
    out = np.empty((T, C), dtype=np.float32)
    for h in range(H):
        out[:, CH * h:CH * (h + 1)] = res.results[h]["outT"]
    return out.reshape(spatial + (C,))


# revision 21
# speedup vs baseline: 1.0270x; 1.0270x over previous
"""Trainium2 Bass kernel for nn_AttentionBlock (GroupNorm + QKV + MHA).

Sharding: tensor-parallel over the H=8 heads, one head per NeuronCore.
Each core:
  - loads x^T (C on partitions) once, in bf16 (halves DMA + stats time),
  - computes GroupNorm statistics with bn_stats + tiny indicator matmuls,
  - folds the (data-dependent) GroupNorm affine INTO the QKV weights:
      xn = x*A + B  =>  qkv = x @ (A*W) + (b + B@W)
  - computes Q^T/K^T packed as [Q;K]; the swapped [K;Q] layout needed for
    concurrent 64x128 row-tile QK^T matmuls (tile_position) is produced by
    two SBUF->SBUF DMA copies per chunk instead of a second projection,
  - softmax without max-subtraction (scores are O(1) here), denominator via
    a ones-column appended to V in the P^T-orientation AV matmul,
  - epilogue transposes the [ch, t] PSUM output to token-major via the PE,
    so the softmax denominator lands spread over 128 partitions and the
    reciprocal + normalization are cheap per-partition DVE ops.
Host side only reorders/permutes arrays and concatenates per-head outputs.
"""
import sys

try:
    import concourse.bass as bass  # noqa: F401
except ImportError:
    sys.path.insert(0, "/opt/trn_rl_repo")
    import concourse.bass as bass  # noqa: F401

import numpy as np
import ml_dtypes
import concourse.tile as tile
from concourse import bacc, mybir
from concourse.bass_utils import run_bass_kernel_spmd

F32 = mybir.dt.float32
F32R = mybir.dt.float32r
F16 = mybir.dt.float16
BF16 = mybir.dt.bfloat16
AF = mybir.ActivationFunctionType
OP = mybir.AluOpType

T = 4096          # tokens (64*64)
C = 512           # channels
H = 8             # heads
CH = C // H       # 64 head dim
G = 32            # groupnorm groups
EPS = 1e-6
QKS = float(CH) ** -0.25   # scale applied to q and k each
NCORES = 8
TC = 512          # t-chunk width
NTC = T // TC     # 8 chunks
NSB = T // 128    # 32 s-blocks
NWARM = 28        # PE warmup matmuls
DEBUG_DUMP = False # add debug DRAM dumps of intermediates


def build_nc():
    nc = bacc.Bacc()
    xt = nc.declare_dram_parameter("xt", [C, T], BF16, isOutput=False)
    wqk = nc.declare_dram_parameter("wqk", [C, 128], BF16, isOutput=False)
    wv = nc.declare_dram_parameter("wv", [C, CH], BF16, isOutput=False)
    bqk = nc.declare_dram_parameter("bqk", [128, 1], F32, isOutput=False)
    bvT = nc.declare_dram_parameter("bvT", [CH, 1], F32, isOutput=False)
    gns = nc.declare_dram_parameter("gns", [128, 4], F32, isOutput=False)
    gnb = nc.declare_dram_parameter("gnb", [128, 4], F32, isOutput=False)
    ind = nc.declare_dram_parameter("ind", [128, 8], F32R, isOutput=False)
    indt = nc.declare_dram_parameter("indt", [8, 128], F32R, isOutput=False)
    outT = nc.declare_dram_parameter("outT", [T, CH], F32, isOutput=True)
    dbg = None
    if DEBUG_DUMP:
        dbg = {
            "dbg_vt": nc.declare_dram_parameter("dbg_vt", [128, NSB * 80], F16, isOutput=True),
            "dbg_qk1": nc.declare_dram_parameter("dbg_qk1", [128, T], F16, isOutput=True),
            "dbg_kq2": nc.declare_dram_parameter("dbg_kq2", [128, T], F16, isOutput=True),
        }

    with tile.TileContext(nc) as tc:
        _build_body(nc, tc, xt, wqk, wv, bqk, bvT, gns, gnb, ind, indt, outT,
                    dbg)
    nc.finalize()
    return nc


def _build_body(nc, tc, xt, wqk, wv, bqk, bvT, gns, gnb, ind, indt, outT,
                dbg=None):
    from contextlib import ExitStack
    ctx = ExitStack()
    with ctx:
        const = ctx.enter_context(tc.tile_pool(name="const", bufs=1))
        big = ctx.enter_context(tc.tile_pool(name="big", bufs=1))
        work = ctx.enter_context(tc.tile_pool(name="work", bufs=2))
        epi = ctx.enter_context(tc.tile_pool(name="epi", bufs=2))
        ppool = ctx.enter_context(tc.tile_pool(name="ppool", bufs=6))
        ps_pair = ctx.enter_context(tc.tile_pool(name="ps_pair", bufs=2, space="PSUM"))
        ps_o = ctx.enter_context(tc.tile_pool(name="ps_o", bufs=2, space="PSUM"))
        ps_sm = ps_o

        # ---------------- loads ----------------
        # weights first (small) so the PE warmup can start immediately
        wqk_raw, wv_raw = [], []
        for k in range(4):
            a = const.tile([128, 128], BF16, tag=f"wqkr{k}")
            nc.sync.dma_start(out=a, in_=wqk[128 * k:128 * (k + 1), :])
            wqk_raw.append(a)
            v = const.tile([128, CH], BF16, tag=f"wvr{k}")
            nc.sync.dma_start(out=v, in_=wv[128 * k:128 * (k + 1), :])
            wv_raw.append(v)
        # whole-tile x loads (8KB contiguous rows aggregate best); arrival
        # order 0,2,1,3 so the DVE (tiles 0,1,3) and ACT (tile 2) stats
        # pipelines both start as early as possible.
        xtk = [None] * 4
        for k in (0, 2, 1, 3):
            t = big.tile([128, T], BF16, tag=f"xt{k}")
            nc.sync.dma_start(out=t, in_=xt[128 * k:128 * (k + 1), :])
            xtk[k] = t
        bqk_t = const.tile([128, 1], F32, tag="bqk")
        nc.sync.dma_start(out=bqk_t, in_=bqk[:, :])
        bvT_t = const.tile([CH, 1], F32, tag="bvT")
        nc.sync.dma_start(out=bvT_t, in_=bvT[:, :])
        gns_t = const.tile([128, 4], F32, tag="gns")
        nc.sync.dma_start(out=gns_t, in_=gns[:, :])
        gnb_t = const.tile([128, 4], F32, tag="gnb")
        nc.sync.dma_start(out=gnb_t, in_=gnb[:, :])
        ind_t = const.tile([128, 8], F32R, tag="ind")
        nc.sync.dma_start(out=ind_t, in_=ind[:, :])
        indt_t = const.tile([8, 128], F32R, tag="indt")
        nc.sync.dma_start(out=indt_t, in_=indt[:, :])

        # f16 identity for PE transposes (V^T -> vt, epilogue -> token-major)
        ident = const.tile([128, 128], F16, tag="ident")
        from concourse.masks import make_identity
        make_identity(nc, ident)

        # PE warmup: dummy matmuls paced by the arriving xt quarters keep
        # the HAM activity monitor warm through the DMA+stats startup so
        # phase B starts at 2.4 GHz instead of 1.2 GHz. Results unused.
        warm_ps = ps_sm.tile([128, TC], F32, name="warm_ps", tag="psO")
        for w_i in range(NWARM):
            k = w_i % 4
            q = (w_i // 4) % 4
            nc.tensor.matmul(out=warm_ps, lhsT=wqk_raw[k],
                             rhs=xtk[k][:, bass.ts(q, 1024)][:, 0:512],
                             start=True, stop=True)

        # ---------------- phase A: groupnorm stats -> folded weights ------
        # DVE bn_stats handles k-tiles {0,1,3}; the (otherwise idle) ACT
        # engine handles tile 2 via accum_out running sums of x and x^2.
        RS = work.tile([128, 8], F32R, tag="RS")
        tmp1 = work.tile([128, 1], F32, tag="tmp1")
        scr = big.tile([128, T], BF16, tag="scr")
        sx = work.tile([128, 1], F32, tag="sx")
        sx2 = work.tile([128, 1], F32, tag="sx2")
        for k in range(4):
            if k == 2:
                nc.scalar.activation(out=scr, in_=xtk[k], func=AF.Identity,
                                     accum_out=sx)
                nc.scalar.activation(out=scr, in_=xtk[k], func=AF.Square,
                                     accum_out=sx2)
                nc.vector.tensor_scalar_mul(out=RS[:, 2 * k:2 * k + 1],
                                            in0=sx, scalar1=1.0 / T)
                nc.vector.tensor_scalar_mul(out=RS[:, 2 * k + 1:2 * k + 2],
                                            in0=sx2, scalar1=1.0 / T)
                continue
            st = work.tile([128, 8, 6], F32, tag="st")
            for sub in range(8):
                nc.vector.bn_stats(out=st[:, sub, :],
                                   in_=xtk[k][:, 512 * sub:512 * (sub + 1)])
            mv = work.tile([128, 2], F32, tag="mv")
            nc.vector.bn_aggr(out=mv, in_=st)
            # RS[:, 2k] = mean ; RS[:, 2k+1] = var + mean^2 = E[x^2]
            nc.vector.tensor_copy(out=RS[:, 2 * k:2 * k + 1], in_=mv[:, 0:1])
            nc.vector.tensor_mul(out=tmp1, in0=mv[:, 0:1], in1=mv[:, 0:1])
            nc.vector.tensor_add(out=RS[:, 2 * k + 1:2 * k + 2], in0=tmp1,
                                 in1=mv[:, 1:2])
        psG = ps_sm.tile([128, 8], F32, tag="psO")
        nc.tensor.matmul(out=psG[0:8, :], lhsT=ind_t, rhs=RS, start=True,
                         stop=True)
        # group stats (8 groups-per-tile x 4 tiles)
        gm = work.tile([8, 4], F32, tag="gm")
        nc.vector.tensor_scalar_mul(out=gm, in0=psG[0:8, 0:8:2], scalar1=1.0 / 16)
        ge2 = work.tile([8, 4], F32, tag="ge2")
        nc.vector.tensor_scalar_mul(out=ge2, in0=psG[0:8, 1:8:2], scalar1=1.0 / 16)
        gv = work.tile([8, 4], F32, tag="gv")
        t8 = work.tile([8, 4], F32, tag="t8")
        nc.vector.tensor_mul(out=t8, in0=gm, in1=gm)
        nc.vector.tensor_sub(out=gv, in0=ge2, in1=t8)
        # rstd = rsqrt(var+eps), one Newton step for accuracy
        gve = work.tile([8, 4], F32, tag="gve")
        nc.vector.tensor_scalar_add(out=gve, in0=gv, scalar1=EPS)
        sq0 = work.tile([8, 4], F32, tag="sq0")
        nc.scalar.activation(out=sq0, in_=gve, func=AF.Sqrt)
        r0 = work.tile([8, 4], F32, tag="r0")
        nc.vector.reciprocal(out=r0, in_=sq0)
        nc.vector.tensor_mul(out=t8, in0=r0, in1=r0)
        nc.vector.tensor_mul(out=t8, in0=t8, in1=gve)
        nc.vector.tensor_scalar(out=t8, in0=t8, scalar1=-0.5, scalar2=1.5,
                                op0=OP.mult, op1=OP.add)
        grstd = work.tile([8, 4], F32, tag="grstd")
        nc.vector.tensor_mul(out=grstd, in0=r0, in1=t8)
        # G[:, 2k] = gmean_k ; G[:, 2k+1] = grstd_k
        Gt = work.tile([8, 8], F32R, tag="Gt")
        nc.vector.tensor_copy(out=Gt[:, 0:8:2], in_=gm)
        nc.vector.tensor_copy(out=Gt[:, 1:8:2], in_=grstd)
        psB = ps_sm.tile([128, 8], F32, tag="psO")
        nc.tensor.matmul(out=psB, lhsT=indt_t, rhs=Gt, start=True, stop=True)
        # per-channel A = rstd*gn_scale ; B = gn_bias - mean*A
        Av = work.tile([128, 4], F32, tag="Av")
        nc.vector.tensor_mul(out=Av, in0=psB[:, 1:8:2], in1=gns_t)
        t128 = work.tile([128, 4], F32, tag="t128")
        nc.vector.tensor_mul(out=t128, in0=psB[:, 0:8:2], in1=Av)
        Bc = work.tile([128, 4], F32, tag="Bc")
        nc.vector.tensor_sub(out=Bc, in0=gnb_t, in1=t128)
        Br = work.tile([128, 4], F32, tag="Br")
        nc.vector.tensor_copy(out=Br, in_=Bc)
        Bsr = work.tile([128, 4], F32, tag="Bsr")
        nc.vector.tensor_scalar_mul(out=Bsr, in0=Bc, scalar1=QKS)
        # column-duplicated copies: fp32r matmuls need an even moving dim
        Bsr2 = work.tile([128, 8], BF16, tag="Bsr2")
        nc.vector.tensor_copy(out=Bsr2[:, 0:8:2], in_=Bsr)
        nc.vector.tensor_copy(out=Bsr2[:, 1:8:2], in_=Bsr)
        Br2 = work.tile([128, 8], BF16, tag="Br2")
        nc.vector.tensor_copy(out=Br2[:, 0:8:2], in_=Br)
        nc.vector.tensor_copy(out=Br2[:, 1:8:2], in_=Br)
        Aqk = work.tile([128, 4], F32, tag="Aqk")
        nc.vector.tensor_scalar_mul(out=Aqk, in0=Av, scalar1=QKS)
        # scaled weights, cast to bf16 to match the bf16 x
        wqk_s, wv_s = [], []
        for k in range(4):
            a = const.tile([128, 128], BF16, tag=f"wqks{k}")
            nc.vector.tensor_scalar_mul(out=a, in0=wqk_raw[k],
                                        scalar1=Aqk[:, k:k + 1])
            wqk_s.append(a)
            v = const.tile([128, CH], BF16, tag=f"wvs{k}")
            nc.vector.tensor_scalar_mul(out=v, in0=wv_raw[k],
                                        scalar1=Av[:, k:k + 1])
            wv_s.append(v)
        # folded biases: b'' = s*b_host + (s*B) @ W_raw   (qk)  /  b + B @ Wv
        psQKb = ps_sm.tile([128, 2], F32, tag="psO")
        for k in range(4):
            nc.tensor.matmul(out=psQKb, lhsT=wqk_raw[k],
                             rhs=Bsr2[:, 2 * k:2 * k + 2],
                             start=(k == 0), stop=(k == 3))
        bqk_dev = const.tile([128, 1], F32, tag="bqk_dev")
        nc.vector.tensor_add(out=bqk_dev, in0=psQKb[:, 0:1], in1=bqk_t)
        psBvT = ps_sm.tile([CH, 2], F32, tag="psO")
        for k in range(4):
            nc.tensor.matmul(out=psBvT, lhsT=wv_raw[k],
                             rhs=Br2[:, 2 * k:2 * k + 2],
                             start=(k == 0), stop=(k == 3))
        bv_dev = const.tile([CH, 1], F32, tag="bv_dev")
        nc.vector.tensor_add(out=bv_dev, in0=psBvT[:, 0:1], in1=bvT_t)

        # ---------------- phase B: projections ----------------
        qk1 = big.tile([128, T], F16, tag="qk1")   # [Q; K]
        kq2 = big.tile([128, T], F16, tag="kq2")   # [K; Q] via DMA copies

        for cc in range(NTC):
            sl = bass.ts(cc, TC)
            ps1 = ps_pair.tile([128, 1536], F32, tag="pp")
            for k in range(4):
                nc.tensor.matmul(out=ps1[:, 0:512], lhsT=wqk_s[k],
                                 rhs=xtk[k][:, sl], start=(k == 0),
                                 stop=(k == 3))
            # bias-add on ACT (idle during this phase), writes f16
            nc.scalar.activation(out=qk1[:, sl], in_=ps1[:, 0:512],
                                 func=AF.Identity, bias=bqk_dev, scale=1.0)
            # swapped layout via SBUF->SBUF DMA (partition-crossing copy)
            nc.sync.dma_start(out=kq2[0:64, sl], in_=qk1[64:128, sl])
            nc.sync.dma_start(out=kq2[64:128, sl], in_=qk1[0:64, sl])
        # V^T projection, then DMA-transpose (XBAR) into token-major vt
        # (with a ones column for the softmax denominator). The transposes
        # run on the idle DMA engines instead of the PE.
        vts = big.tile([CH, T], F16, tag="vts")
        # inner stride 80: XBAR-transpose output rows need 32-byte alignment
        vt = big.tile([128, NSB, 80], F16, tag="vt")
        nc.vector.memset(vt[:, :, CH:CH + 1], 1.0)

        def vts_block():
            # emitted inside chunk 0, after its first few QK^T pairs, so the
            # early softmax exps overlap the V projection on the ACT engine
            for vc in range(NTC):
                vsl = bass.ts(vc, TC)
                psVT = ps_sm.tile([CH, TC], F32, name=f"psVT_{vc}", tag="psO")
                for k in range(4):
                    nc.tensor.matmul(out=psVT, lhsT=wv_s[k],
                                     rhs=xtk[k][:, vsl],
                                     start=(k == 0), stop=(k == 3))
                nc.scalar.activation(out=vts[:, vsl], in_=psVT,
                                     func=AF.Identity, bias=bv_dev, scale=1.0)
                # one XBAR transpose per chunk: out[p,j,c] = vts[c, j*128+p]
                nc.sync.dma_start(out=vt[:, 4 * vc:4 * vc + 4, 0:CH],
                                  in_=vts[:, vsl], transpose=True)

        # ---------------- phase C: attention ----------------
        # S^T tiles grouped 3 s-blocks wide so each ACT exp covers 1536
        # columns (amortizes per-instruction overhead on the bottleneck
        # engine). mm1 runs as concurrent 64x128 row-tile pairs.
        NP = NSB // 2
        NT_C = 11  # 10 tiles of 3 blocks + 1 tile of 2 blocks

        def epi_finish(state):
            # psO holds [out^T; den] (65 x 512). Transpose to token-major so
            # the denominator spreads across partitions, then the reciprocal
            # and the normalizing multiply are cheap per-partition DVE ops.
            cc0, sAB0 = state
            # inner dim padded to CH+2 so each j-slice is 4-byte aligned in PSUM
            psT = ps_o.tile([128, 4, CH + 2], F16, name=f"psT_{cc0}", tag="psO")
            for j in range(4):
                nc.tensor.transpose(out=psT[:, j, 0:CH + 1],
                                    in_=sAB0[:, bass.ts(j, 128)],
                                    identity=ident[0:CH + 1, 0:CH + 1])
            rden = work.tile([128, 4], F32, name=f"rden_{cc0}", tag="rden")
            nc.vector.reciprocal(out=rden, in_=psT[:, :, CH])
            osb = work.tile([128, 4, CH], F32, name=f"osb_{cc0}", tag="osb")
            for j in range(4):
                nc.vector.tensor_scalar_mul(out=osb[:, j, :],
                                            in0=psT[:, j, 0:CH],
                                            scalar1=rden[:, j:j + 1])
                nc.sync.dma_start(
                    out=outT[TC * cc0 + 128 * j:TC * cc0 + 128 * (j + 1), :],
                    in_=osb[:, j, :])

        def mm1_pair(cc, pr, tiles):
            sl = bass.ts(cc, TC)
            for half, blk in ((0, 2 * pr), (1, 2 * pr + 1)):
                ti, col = divmod(blk, 3)
                dst = tiles[ti][1][:, 512 * col:512 * (col + 1)]
                if half == 0:
                    nc.tensor.matmul(out=dst,
                                     lhsT=kq2[0:64, bass.ts(blk, 128)],
                                     rhs=qk1[0:64, sl], start=True, stop=True,
                                     tile_position=(0, 0))
                else:
                    nc.tensor.matmul(out=dst,
                                     lhsT=qk1[64:128, bass.ts(blk, 128)],
                                     rhs=kq2[64:128, sl], start=True,
                                     stop=True, tile_position=(64, 0))

        def exp_tile(tiles, ti):
            nblk, pp = tiles[ti]
            w = 512 * nblk
            P = ppool.tile([128, 1536], F16, tag="P")
            nc.scalar.activation(out=P[:, 0:w], in_=pp[:, 0:w], func=AF.Exp)
            return P

        def mm2_tile(psO, ti, P, nblk, first, last):
            for c in range(nblk):
                blk = 3 * ti + c
                nc.tensor.matmul(out=psO[0:CH + 1, :],
                                 lhsT=vt[:, blk, 0:CH + 1],
                                 rhs=P[:, 512 * c:512 * (c + 1)],
                                 start=(first and c == 0),
                                 stop=(last and c == nblk - 1))

        pending = None
        for cc in range(NTC):
            # chunk 0: psO allocation deferred past the V block so the ps_o
            # PSUM ring stays strictly sequential-use
            psO = None
            if cc > 0:
                psO = ps_o.tile([128, TC], F32, name=f"psO_{cc}", tag="psO")
            mm2_upto = 0
            tiles = {}
            done_upto = -1
            flushed = []   # (ti, P, nblk)
            for pr in range(NP):
                for blk in (2 * pr, 2 * pr + 1):
                    ti = blk // 3
                    if ti not in tiles:
                        nblk = 2 if ti == NT_C - 1 else 3
                        tiles[ti] = (nblk, ps_pair.tile(
                            [128, 1536], F32, name=f"pp_{cc}_{ti}", tag="pp"))
                mm1_pair(cc, pr, tiles)
                if cc == 0 and pr == 5:
                    vts_block()
                    psO = ps_o.tile([128, TC], F32, name="psO_0", tag="psO")
                pair_done = 2 * pr + 1
                while True:
                    ti = done_upto + 1
                    if ti >= NT_C or ti not in tiles:
                        break
                    nblk = tiles[ti][0]
                    if 3 * ti + nblk - 1 > pair_done:
                        break
                    P = exp_tile(tiles, ti)
                    flushed.append((ti, P, nblk))
                    done_upto = ti
                    if pending is not None and ti == 2:
                        epi_finish(pending)
                        pending = None
                if psO is not None:
                    while mm2_upto < len(flushed) - 1:
                        pti, pP, pn = flushed[mm2_upto]
                        mm2_tile(psO, pti, pP, pn, first=(pti == 0),
                                 last=False)
                        mm2_upto += 1
            lti, lP, ln = flushed[-1]
            mm2_tile(psO, lti, lP, ln, first=False, last=True)
            if cc == 0 and dbg is not None:
                nc.sync.dma_start(out=dbg["dbg_vt"][:, :],
                                  in_=vt[:, :, :])
                nc.sync.dma_start(out=dbg["dbg_qk1"][:, :], in_=qk1[:, :])
                nc.sync.dma_start(out=dbg["dbg_kq2"][:, :], in_=kq2[:, :])
            # pull [out^T; den] into SBUF (f16) so the PE can transpose it;
            # this also frees psO's PSUM slot early for the ring.
            sAB = epi.tile([CH + 1, TC], F16, name=f"sAB_{cc}", tag="sAB")
            nc.vector.tensor_copy(out=sAB, in_=psO[0:CH + 1, :])
            pending = (cc, sAB)
        epi_finish(pending)


_NC_CACHE = None
TRACE = False          # set True (e.g. from test.py) to capture an NTFF profile
TMPDIR = None          # set to a path to keep NEFF/NTFF artifacts
LAST_RESULT = None     # BassKernelResults of the most recent kernel() call


def _get_nc():
    global _NC_CACHE
    if _NC_CACHE is None:
        _NC_CACHE = build_nc()
    return _NC_CACHE


def kernel(x, gn_scale, gn_bias, w_qkv, b_qkv):
    x = np.asarray(x, dtype=np.float32)
    gn_scale = np.asarray(gn_scale, dtype=np.float32)
    gn_bias = np.asarray(gn_bias, dtype=np.float32)
    w_qkv = np.asarray(w_qkv, dtype=np.float32)
    b_qkv = np.asarray(b_qkv, dtype=np.float32)

    spatial = x.shape[:-1]
    xt = np.ascontiguousarray(x.reshape(T, C).T).astype(ml_dtypes.bfloat16)
    w3 = w_qkv.reshape(C, C, 3)
    wq, wk, wv = w3[..., 0], w3[..., 1], w3[..., 2]          # (C, C) each
    b3 = b_qkv.reshape(C, 3)
    bq, bk, bvv = b3[:, 0], b3[:, 1], b3[:, 2]

    # indicator matrices: channel partition p -> group j = p // 16
    p_idx = np.arange(128)
    ind = (p_idx[:, None] // 16 == np.arange(8)[None, :]).astype(np.float32)
    indt = np.ascontiguousarray(ind.T)
    gns_dev = np.ascontiguousarray(gn_scale.reshape(4, 128).T)
    gnb_dev = np.ascontiguousarray(gn_bias.reshape(4, 128).T)

    in_maps = []
    for h in range(H):
        hsl = slice(CH * h, CH * (h + 1))
        wq_h, wk_h, wv_h = wq[:, hsl], wk[:, hsl], wv[:, hsl]
        in_maps.append({
            "xt": xt,
            "wqk": np.concatenate([wq_h, wk_h], axis=1).astype(ml_dtypes.bfloat16),
            "wv": np.ascontiguousarray(wv_h).astype(ml_dtypes.bfloat16),
            "bqk": (QKS * np.concatenate([bq[hsl], bk[hsl]])).reshape(128, 1),
            "bvT": bvv[hsl].reshape(CH, 1).copy(),
            "gns": gns_dev,
            "gnb": gnb_dev,
            "ind": ind,
            "indt": indt,
        })

    nc = _get_nc()
    res = run_bass_kernel_spmd(nc, in_maps, list(range(NCORES)), trace=TRACE,
                               tmpdir=TMPDIR)
    global LAST_RESULT
    LAST_RESULT = res
    out = np.empty((T, C), dtype=np.float32)
    for h in range(H):
        out[:, CH * h:CH * (h + 1)] = res.results[h]["outT"]
    return out.reshape(spatial + (C,))


# revision 22
# speedup vs baseline: 1.0515x; 1.0239x over previous
"""Trainium2 Bass kernel for nn_AttentionBlock (GroupNorm + QKV + MHA).

Sharding: tensor-parallel over the H=8 heads, one head per NeuronCore.
Each core:
  - loads x^T (C on partitions) once, in bf16 (halves DMA + stats time),
  - computes GroupNorm statistics with bn_stats + tiny indicator matmuls,
  - folds the (data-dependent) GroupNorm affine INTO the QKV weights:
      xn = x*A + B  =>  qkv = x @ (A*W) + (b + B@W)
  - computes Q^T/K^T packed as [Q;K]; the swapped [K;Q] layout needed for
    concurrent 64x128 row-tile QK^T matmuls (tile_position) is produced by
    two SBUF->SBUF DMA copies per chunk instead of a second projection,
  - softmax without max-subtraction (scores are O(1) here), denominator via
    a ones-column appended to V in the P^T-orientation AV matmul,
  - epilogue transposes the [ch, t] PSUM output to token-major via the PE,
    so the softmax denominator lands spread over 128 partitions and the
    reciprocal + normalization are cheap per-partition DVE ops.
Host side only reorders/permutes arrays and concatenates per-head outputs.
"""
import sys

try:
    import concourse.bass as bass  # noqa: F401
except ImportError:
    sys.path.insert(0, "/opt/trn_rl_repo")
    import concourse.bass as bass  # noqa: F401

import numpy as np
import ml_dtypes
import concourse.tile as tile
from concourse import bacc, mybir
from concourse.bass_utils import run_bass_kernel_spmd

F32 = mybir.dt.float32
F32R = mybir.dt.float32r
F16 = mybir.dt.float16
BF16 = mybir.dt.bfloat16
AF = mybir.ActivationFunctionType
OP = mybir.AluOpType

T = 4096          # tokens (64*64)
C = 512           # channels
H = 8             # heads
CH = C // H       # 64 head dim
G = 32            # groupnorm groups
EPS = 1e-6
QKS = float(CH) ** -0.25   # scale applied to q and k each
NCORES = 8
TC = 512          # t-chunk width
NTC = T // TC     # 8 chunks
NSB = T // 128    # 32 s-blocks
NWARM = 28        # PE warmup matmuls
DEBUG_DUMP = False # add debug DRAM dumps of intermediates


def build_nc():
    nc = bacc.Bacc()
    xt = nc.declare_dram_parameter("xt", [C, T], BF16, isOutput=False)
    wqk = nc.declare_dram_parameter("wqk", [C, 128], BF16, isOutput=False)
    wv = nc.declare_dram_parameter("wv", [C, CH], BF16, isOutput=False)
    bqk = nc.declare_dram_parameter("bqk", [128, 1], F32, isOutput=False)
    bvT = nc.declare_dram_parameter("bvT", [CH, 1], F32, isOutput=False)
    gns = nc.declare_dram_parameter("gns", [128, 4], F32, isOutput=False)
    gnb = nc.declare_dram_parameter("gnb", [128, 4], F32, isOutput=False)
    ind = nc.declare_dram_parameter("ind", [128, 8], F32R, isOutput=False)
    indt = nc.declare_dram_parameter("indt", [8, 128], F32R, isOutput=False)
    outT = nc.declare_dram_parameter("outT", [T, CH], F32, isOutput=True)
    dbg = None
    if DEBUG_DUMP:
        dbg = {
            "dbg_vt": nc.declare_dram_parameter("dbg_vt", [128, NSB * 80], F16, isOutput=True),
            "dbg_qk1": nc.declare_dram_parameter("dbg_qk1", [128, T], F16, isOutput=True),
            "dbg_kq2": nc.declare_dram_parameter("dbg_kq2", [128, T], F16, isOutput=True),
        }

    with tile.TileContext(nc) as tc:
        _build_body(nc, tc, xt, wqk, wv, bqk, bvT, gns, gnb, ind, indt, outT,
                    dbg)
    nc.finalize()
    return nc


def _build_body(nc, tc, xt, wqk, wv, bqk, bvT, gns, gnb, ind, indt, outT,
                dbg=None):
    from contextlib import ExitStack
    ctx = ExitStack()
    with ctx:
        const = ctx.enter_context(tc.tile_pool(name="const", bufs=1))
        big = ctx.enter_context(tc.tile_pool(name="big", bufs=1))
        work = ctx.enter_context(tc.tile_pool(name="work", bufs=2))
        epi = ctx.enter_context(tc.tile_pool(name="epi", bufs=2))
        ppool = ctx.enter_context(tc.tile_pool(name="ppool", bufs=6))
        ps_pair = ctx.enter_context(tc.tile_pool(name="ps_pair", bufs=2, space="PSUM"))
        ps_o = ctx.enter_context(tc.tile_pool(name="ps_o", bufs=2, space="PSUM"))
        ps_sm = ps_o

        # ---------------- loads ----------------
        # weights first (small) so the PE warmup can start immediately
        wqk_raw, wv_raw = [], []
        for k in range(4):
            a = const.tile([128, 128], BF16, tag=f"wqkr{k}")
            nc.sync.dma_start(out=a, in_=wqk[128 * k:128 * (k + 1), :])
            wqk_raw.append(a)
            v = const.tile([128, CH], BF16, tag=f"wvr{k}")
            nc.sync.dma_start(out=v, in_=wv[128 * k:128 * (k + 1), :])
            wv_raw.append(v)
        # whole-tile x loads (8KB contiguous rows aggregate best); arrival
        # order 0,2,1,3 so the DVE (tiles 0,1,3) and ACT (tile 2) stats
        # pipelines both start as early as possible.
        xtk = [None] * 4
        for k in (0, 2, 1, 3):
            t = big.tile([128, T], BF16, tag=f"xt{k}")
            nc.sync.dma_start(out=t, in_=xt[128 * k:128 * (k + 1), :])
            xtk[k] = t
        bqk_t = const.tile([128, 1], F32, tag="bqk")
        nc.sync.dma_start(out=bqk_t, in_=bqk[:, :])
        bvT_t = const.tile([CH, 1], F32, tag="bvT")
        nc.sync.dma_start(out=bvT_t, in_=bvT[:, :])
        gns_t = const.tile([128, 4], F32, tag="gns")
        nc.sync.dma_start(out=gns_t, in_=gns[:, :])
        gnb_t = const.tile([128, 4], F32, tag="gnb")
        nc.sync.dma_start(out=gnb_t, in_=gnb[:, :])
        ind_t = const.tile([128, 8], F32R, tag="ind")
        nc.sync.dma_start(out=ind_t, in_=ind[:, :])
        indt_t = const.tile([8, 128], F32R, tag="indt")
        nc.sync.dma_start(out=indt_t, in_=indt[:, :])

        # f16 identity for PE transposes (V^T -> vt, epilogue -> token-major)
        ident = const.tile([128, 128], F16, tag="ident")
        from concourse.masks import make_identity
        make_identity(nc, ident)

        # PE warmup: dummy matmuls paced by the arriving xt quarters keep
        # the HAM activity monitor warm through the DMA+stats startup so
        # phase B starts at 2.4 GHz instead of 1.2 GHz. Results unused.
        warm_ps = ps_sm.tile([128, TC], F32, name="warm_ps", tag="psO")
        for w_i in range(NWARM):
            k = w_i % 4
            q = (w_i // 4) % 4
            nc.tensor.matmul(out=warm_ps, lhsT=wqk_raw[k],
                             rhs=xtk[k][:, bass.ts(q, 1024)][:, 0:512],
                             start=True, stop=True)

        # ---------------- phase A: groupnorm stats -> folded weights ------
        # DVE bn_stats handles k-tiles {0,1,3}; the (otherwise idle) ACT
        # engine handles tile 2 via accum_out running sums of x and x^2.
        RS = work.tile([128, 8], F32R, tag="RS")
        tmp1 = work.tile([128, 1], F32, tag="tmp1")
        scr = big.tile([128, T], BF16, tag="scr")
        sx = work.tile([128, 1], F32, tag="sx")
        sx2 = work.tile([128, 1], F32, tag="sx2")
        for k in range(4):
            if k == 2:
                nc.scalar.activation(out=scr, in_=xtk[k], func=AF.Identity,
                                     accum_out=sx)
                nc.scalar.activation(out=scr, in_=xtk[k], func=AF.Square,
                                     accum_out=sx2)
                nc.vector.tensor_scalar_mul(out=RS[:, 2 * k:2 * k + 1],
                                            in0=sx, scalar1=1.0 / T)
                nc.vector.tensor_scalar_mul(out=RS[:, 2 * k + 1:2 * k + 2],
                                            in0=sx2, scalar1=1.0 / T)
                continue
            st = work.tile([128, 8, 6], F32, tag="st")
            for sub in range(8):
                nc.vector.bn_stats(out=st[:, sub, :],
                                   in_=xtk[k][:, 512 * sub:512 * (sub + 1)])
            mv = work.tile([128, 2], F32, tag="mv")
            nc.vector.bn_aggr(out=mv, in_=st)
            # RS[:, 2k] = mean ; RS[:, 2k+1] = var + mean^2 = E[x^2]
            nc.vector.tensor_copy(out=RS[:, 2 * k:2 * k + 1], in_=mv[:, 0:1])
            nc.vector.tensor_mul(out=tmp1, in0=mv[:, 0:1], in1=mv[:, 0:1])
            nc.vector.tensor_add(out=RS[:, 2 * k + 1:2 * k + 2], in0=tmp1,
                                 in1=mv[:, 1:2])
        psG = ps_sm.tile([128, 8], F32, tag="psO")
        nc.tensor.matmul(out=psG[0:8, :], lhsT=ind_t, rhs=RS, start=True,
                         stop=True)
        # group stats (8 groups-per-tile x 4 tiles)
        gm = work.tile([8, 4], F32, tag="gm")
        nc.vector.tensor_scalar_mul(out=gm, in0=psG[0:8, 0:8:2], scalar1=1.0 / 16)
        ge2 = work.tile([8, 4], F32, tag="ge2")
        nc.vector.tensor_scalar_mul(out=ge2, in0=psG[0:8, 1:8:2], scalar1=1.0 / 16)
        gv = work.tile([8, 4], F32, tag="gv")
        t8 = work.tile([8, 4], F32, tag="t8")
        nc.vector.tensor_mul(out=t8, in0=gm, in1=gm)
        nc.vector.tensor_sub(out=gv, in0=ge2, in1=t8)
        # rstd = rsqrt(var+eps), one Newton step for accuracy
        gve = work.tile([8, 4], F32, tag="gve")
        nc.vector.tensor_scalar_add(out=gve, in0=gv, scalar1=EPS)
        sq0 = work.tile([8, 4], F32, tag="sq0")
        nc.scalar.activation(out=sq0, in_=gve, func=AF.Sqrt)
        r0 = work.tile([8, 4], F32, tag="r0")
        nc.vector.reciprocal(out=r0, in_=sq0)
        nc.vector.tensor_mul(out=t8, in0=r0, in1=r0)
        nc.vector.tensor_mul(out=t8, in0=t8, in1=gve)
        nc.vector.tensor_scalar(out=t8, in0=t8, scalar1=-0.5, scalar2=1.5,
                                op0=OP.mult, op1=OP.add)
        grstd = work.tile([8, 4], F32, tag="grstd")
        nc.vector.tensor_mul(out=grstd, in0=r0, in1=t8)
        # G[:, 2k] = gmean_k ; G[:, 2k+1] = grstd_k
        Gt = work.tile([8, 8], F32R, tag="Gt")
        nc.vector.tensor_copy(out=Gt[:, 0:8:2], in_=gm)
        nc.vector.tensor_copy(out=Gt[:, 1:8:2], in_=grstd)
        psB = ps_sm.tile([128, 8], F32, tag="psO")
        nc.tensor.matmul(out=psB, lhsT=indt_t, rhs=Gt, start=True, stop=True)
        # per-channel A = rstd*gn_scale ; B = gn_bias - mean*A
        Av = work.tile([128, 4], F32, tag="Av")
        nc.vector.tensor_mul(out=Av, in0=psB[:, 1:8:2], in1=gns_t)
        t128 = work.tile([128, 4], F32, tag="t128")
        nc.vector.tensor_mul(out=t128, in0=psB[:, 0:8:2], in1=Av)
        Bc = work.tile([128, 4], F32, tag="Bc")
        nc.vector.tensor_sub(out=Bc, in0=gnb_t, in1=t128)
        Br = work.tile([128, 4], F32, tag="Br")
        nc.vector.tensor_copy(out=Br, in_=Bc)
        Bsr = work.tile([128, 4], F32, tag="Bsr")
        nc.vector.tensor_scalar_mul(out=Bsr, in0=Bc, scalar1=QKS)
        # column-duplicated copies: fp32r matmuls need an even moving dim
        Bsr2 = work.tile([128, 8], BF16, tag="Bsr2")
        nc.vector.tensor_copy(out=Bsr2[:, 0:8:2], in_=Bsr)
        nc.vector.tensor_copy(out=Bsr2[:, 1:8:2], in_=Bsr)
        Br2 = work.tile([128, 8], BF16, tag="Br2")
        nc.vector.tensor_copy(out=Br2[:, 0:8:2], in_=Br)
        nc.vector.tensor_copy(out=Br2[:, 1:8:2], in_=Br)
        Aqk = work.tile([128, 4], F32, tag="Aqk")
        nc.vector.tensor_scalar_mul(out=Aqk, in0=Av, scalar1=QKS)
        # scaled weights, cast to bf16 to match the bf16 x
        wqk_s, wv_s = [], []
        for k in range(4):
            a = const.tile([128, 128], BF16, tag=f"wqks{k}")
            nc.vector.tensor_scalar_mul(out=a, in0=wqk_raw[k],
                                        scalar1=Aqk[:, k:k + 1])
            wqk_s.append(a)
            v = const.tile([128, CH], BF16, tag=f"wvs{k}")
            nc.vector.tensor_scalar_mul(out=v, in0=wv_raw[k],
                                        scalar1=Av[:, k:k + 1])
            wv_s.append(v)
        # folded biases: b'' = s*b_host + (s*B) @ W_raw   (qk)  /  b + B @ Wv
        psQKb = ps_sm.tile([128, 2], F32, tag="psO")
        for k in range(4):
            nc.tensor.matmul(out=psQKb, lhsT=wqk_raw[k],
                             rhs=Bsr2[:, 2 * k:2 * k + 2],
                             start=(k == 0), stop=(k == 3))
        bqk_dev = const.tile([128, 1], F32, tag="bqk_dev")
        nc.vector.tensor_add(out=bqk_dev, in0=psQKb[:, 0:1], in1=bqk_t)
        psBvT = ps_sm.tile([CH, 2], F32, tag="psO")
        for k in range(4):
            nc.tensor.matmul(out=psBvT, lhsT=wv_raw[k],
                             rhs=Br2[:, 2 * k:2 * k + 2],
                             start=(k == 0), stop=(k == 3))
        bv_dev = const.tile([CH, 1], F32, tag="bv_dev")
        nc.vector.tensor_add(out=bv_dev, in0=psBvT[:, 0:1], in1=bvT_t)

        # ---------------- phase B: projections ----------------
        qk1 = big.tile([128, T], F16, tag="qk1")   # [Q; K]
        kq2 = big.tile([128, T], F16, tag="kq2")   # [K; Q] via DMA copies

        for cc in range(NTC):
            sl = bass.ts(cc, TC)
            ps1 = ps_pair.tile([128, 1536], F32, tag="pp")
            for k in range(4):
                nc.tensor.matmul(out=ps1[:, 0:512], lhsT=wqk_s[k],
                                 rhs=xtk[k][:, sl], start=(k == 0),
                                 stop=(k == 3))
            # bias-add on ACT (idle during this phase), writes f16
            nc.scalar.activation(out=qk1[:, sl], in_=ps1[:, 0:512],
                                 func=AF.Identity, bias=bqk_dev, scale=1.0)
            # swapped layout via SBUF->SBUF DMA (partition-crossing copy)
            nc.sync.dma_start(out=kq2[0:64, sl], in_=qk1[64:128, sl])
            nc.sync.dma_start(out=kq2[64:128, sl], in_=qk1[0:64, sl])
        # V^T projection, then DMA-transpose (XBAR) into token-major vt
        # (with a ones column for the softmax denominator). The transposes
        # run on the idle DMA engines instead of the PE.
        vts = big.tile([CH, T], F16, tag="vts")
        # inner stride 80: XBAR-transpose output rows need 32-byte alignment
        vt = big.tile([128, NSB, 80], F16, tag="vt")
        nc.vector.memset(vt[:, :, CH:CH + 1], 1.0)

        def vts_block():
            # emitted inside chunk 0, after its first few QK^T pairs, so the
            # early softmax exps overlap the V projection on the ACT engine
            for vc in range(NTC):
                vsl = bass.ts(vc, TC)
                psVT = ps_sm.tile([CH, TC], F32, name=f"psVT_{vc}", tag="psO")
                for k in range(4):
                    nc.tensor.matmul(out=psVT, lhsT=wv_s[k],
                                     rhs=xtk[k][:, vsl],
                                     start=(k == 0), stop=(k == 3))
                nc.scalar.activation(out=vts[:, vsl], in_=psVT,
                                     func=AF.Identity, bias=bv_dev, scale=1.0)
                # one XBAR transpose per chunk: out[p,j,c] = vts[c, j*128+p]
                nc.sync.dma_start(out=vt[:, 4 * vc:4 * vc + 4, 0:CH],
                                  in_=vts[:, vsl], transpose=True)

        # ---------------- phase C: attention ----------------
        # S^T tiles grouped 3 s-blocks wide so each ACT exp covers 1536
        # columns (amortizes per-instruction overhead on the bottleneck
        # engine). mm1 runs as concurrent 64x128 row-tile pairs.
        NP = NSB // 2
        NT_C = 11  # first tile 2 blocks (early first exp), then 10 of 3

        def tile_of(blk):
            if blk < 2:
                return 0, blk
            return (blk - 2) // 3 + 1, (blk - 2) % 3

        def nblk_of(ti):
            return 2 if ti == 0 else 3

        def last_blk_of(ti):
            return 1 if ti == 0 else 3 * ti + 1

        def epi_finish(state):
            # psO holds [out^T; den] (65 x 512). Transpose to token-major so
            # the denominator spreads across partitions, then the reciprocal
            # and the normalizing multiply are cheap per-partition DVE ops.
            cc0, sAB0 = state
            # inner dim padded to CH+2 so each j-slice is 4-byte aligned in PSUM
            psT = ps_o.tile([128, 4, CH + 2], F16, name=f"psT_{cc0}", tag="psO")
            for j in range(4):
                nc.tensor.transpose(out=psT[:, j, 0:CH + 1],
                                    in_=sAB0[:, bass.ts(j, 128)],
                                    identity=ident[0:CH + 1, 0:CH + 1])
            rden = work.tile([128, 4], F32, name=f"rden_{cc0}", tag="rden")
            nc.vector.reciprocal(out=rden, in_=psT[:, :, CH])
            osb = work.tile([128, 4, CH], F32, name=f"osb_{cc0}", tag="osb")
            for j in range(4):
                nc.vector.tensor_scalar_mul(out=osb[:, j, :],
                                            in0=psT[:, j, 0:CH],
                                            scalar1=rden[:, j:j + 1])
                nc.sync.dma_start(
                    out=outT[TC * cc0 + 128 * j:TC * cc0 + 128 * (j + 1), :],
                    in_=osb[:, j, :])

        def mm1_pair(cc, pr, tiles):
            sl = bass.ts(cc, TC)
            for half, blk in ((0, 2 * pr), (1, 2 * pr + 1)):
                ti, col = tile_of(blk)
                dst = tiles[ti][1][:, 512 * col:512 * (col + 1)]
                if half == 0:
                    nc.tensor.matmul(out=dst,
                                     lhsT=kq2[0:64, bass.ts(blk, 128)],
                                     rhs=qk1[0:64, sl], start=True, stop=True,
                                     tile_position=(0, 0))
                else:
                    nc.tensor.matmul(out=dst,
                                     lhsT=qk1[64:128, bass.ts(blk, 128)],
                                     rhs=kq2[64:128, sl], start=True,
                                     stop=True, tile_position=(64, 0))

        def exp_tile(tiles, ti):
            nblk, pp = tiles[ti]
            w = 512 * nblk
            P = ppool.tile([128, 1536], F16, tag="P")
            nc.scalar.activation(out=P[:, 0:w], in_=pp[:, 0:w], func=AF.Exp)
            return P

        def mm2_tile(psO, ti, P, nblk, first, last):
            for c in range(nblk):
                blk = c if ti == 0 else 3 * ti - 1 + c
                nc.tensor.matmul(out=psO[0:CH + 1, :],
                                 lhsT=vt[:, blk, 0:CH + 1],
                                 rhs=P[:, 512 * c:512 * (c + 1)],
                                 start=(first and c == 0),
                                 stop=(last and c == nblk - 1))

        pending = None
        for cc in range(NTC):
            # chunk 0: psO allocation deferred past the V block so the ps_o
            # PSUM ring stays strictly sequential-use
            psO = None
            if cc > 0:
                psO = ps_o.tile([128, TC], F32, name=f"psO_{cc}", tag="psO")
            mm2_upto = 0
            tiles = {}
            done_upto = -1
            flushed = []   # (ti, P, nblk)
            for pr in range(NP):
                for blk in (2 * pr, 2 * pr + 1):
                    ti = tile_of(blk)[0]
                    if ti not in tiles:
                        tiles[ti] = (nblk_of(ti), ps_pair.tile(
                            [128, 1536], F32, name=f"pp_{cc}_{ti}", tag="pp"))
                mm1_pair(cc, pr, tiles)
                if cc == 0 and pr == 5:
                    vts_block()
                    psO = ps_o.tile([128, TC], F32, name="psO_0", tag="psO")
                pair_done = 2 * pr + 1
                while True:
                    ti = done_upto + 1
                    if ti >= NT_C or ti not in tiles:
                        break
                    nblk = tiles[ti][0]
                    if last_blk_of(ti) > pair_done:
                        break
                    P = exp_tile(tiles, ti)
                    flushed.append((ti, P, nblk))
                    done_upto = ti
                    if pending is not None and ti == 2:
                        epi_finish(pending)
                        pending = None
                if psO is not None:
                    while mm2_upto < len(flushed) - 1:
                        pti, pP, pn = flushed[mm2_upto]
                        mm2_tile(psO, pti, pP, pn, first=(pti == 0),
                                 last=False)
                        mm2_upto += 1
            lti, lP, ln = flushed[-1]
            mm2_tile(psO, lti, lP, ln, first=False, last=True)
            if cc == 0 and dbg is not None:
                nc.sync.dma_start(out=dbg["dbg_vt"][:, :],
                                  in_=vt[:, :, :])
                nc.sync.dma_start(out=dbg["dbg_qk1"][:, :], in_=qk1[:, :])
                nc.sync.dma_start(out=dbg["dbg_kq2"][:, :], in_=kq2[:, :])
            # pull [out^T; den] into SBUF (f16) so the PE can transpose it;
            # this also frees psO's PSUM slot early for the ring.
            sAB = epi.tile([CH + 1, TC], F16, name=f"sAB_{cc}", tag="sAB")
            nc.vector.tensor_copy(out=sAB, in_=psO[0:CH + 1, :])
            pending = (cc, sAB)
        epi_finish(pending)


_NC_CACHE = None
TRACE = False          # set True (e.g. from test.py) to capture an NTFF profile
TMPDIR = None          # set to a path to keep NEFF/NTFF artifacts
LAST_RESULT = None     # BassKernelResults of the most recent kernel() call


def _get_nc():
    global _NC_CACHE
    if _NC_CACHE is None:
        _NC_CACHE = build_nc()
    return _NC_CACHE


def kernel(x, gn_scale, gn_bias, w_qkv, b_qkv):
    x = np.asarray(x, dtype=np.float32)
    gn_scale = np.asarray(gn_scale, dtype=np.float32)
    gn_bias = np.asarray(gn_bias, dtype=np.float32)
    w_qkv = np.asarray(w_qkv, dtype=np.float32)
    b_qkv = np.asarray(b_qkv, dtype=np.float32)

    spatial = x.shape[:-1]
    xt = np.ascontiguousarray(x.reshape(T, C).T).astype(ml_dtypes.bfloat16)
    w3 = w_qkv.reshape(C, C, 3)
    wq, wk, wv = w3[..., 0], w3[..., 1], w3[..., 2]          # (C, C) each
    b3 = b_qkv.reshape(C, 3)
    bq, bk, bvv = b3[:, 0], b3[:, 1], b3[:, 2]

    # indicator matrices: channel partition p -> group j = p // 16
    p_idx = np.arange(128)
    ind = (p_idx[:, None] // 16 == np.arange(8)[None, :]).astype(np.float32)
    indt = np.ascontiguousarray(ind.T)
    gns_dev = np.ascontiguousarray(gn_scale.reshape(4, 128).T)
    gnb_dev = np.ascontiguousarray(gn_bias.reshape(4, 128).T)

    in_maps = []
    for h in range(H):
        hsl = slice(CH * h, CH * (h + 1))
        wq_h, wk_h, wv_h = wq[:, hsl], wk[:, hsl], wv[:, hsl]
        in_maps.append({
            "xt": xt,
            "wqk": np.concatenate([wq_h, wk_h], axis=1).astype(ml_dtypes.bfloat16),
            "wv": np.ascontiguousarray(wv_h).astype(ml_dtypes.bfloat16),
            "bqk": (QKS * np.concatenate([bq[hsl], bk[hsl]])).reshape(128, 1),
            "bvT": bvv[hsl].reshape(CH, 1).copy(),
            "gns": gns_dev,
            "gnb": gnb_dev,
            "ind": ind,
            "indt": indt,
        })

    nc = _get_nc()
    res = run_bass_kernel_spmd(nc, in_maps, list(range(NCORES)), trace=TRACE,
                               tmpdir=TMPDIR)
    global LAST_RESULT
    LAST_RESULT = res
    out = np.empty((T, C), dtype=np.float32)
    for h in range(H):
        out[:, CH * h:CH * (h + 1)] = res.results[h]["outT"]
    return out.reshape(spatial + (C,))


# revision 23
# speedup vs baseline: 1.0584x; 1.0065x over previous
"""Trainium2 Bass kernel for nn_AttentionBlock (GroupNorm + QKV + MHA).

Sharding: tensor-parallel over the H=8 heads, one head per NeuronCore.
Each core:
  - loads x^T (C on partitions) once, in bf16 (halves DMA + stats time),
  - computes GroupNorm statistics with bn_stats + tiny indicator matmuls,
  - folds the (data-dependent) GroupNorm affine INTO the QKV weights:
      xn = x*A + B  =>  qkv = x @ (A*W) + (b + B@W)
  - computes Q^T/K^T packed as [Q;K]; the swapped [K;Q] layout needed for
    concurrent 64x128 row-tile QK^T matmuls (tile_position) is produced by
    two SBUF->SBUF DMA copies per chunk instead of a second projection,
  - softmax without max-subtraction (scores are O(1) here), denominator via
    a ones-column appended to V in the P^T-orientation AV matmul,
  - epilogue transposes the [ch, t] PSUM output to token-major via the PE,
    so the softmax denominator lands spread over 128 partitions and the
    reciprocal + normalization are cheap per-partition DVE ops.
Host side only reorders/permutes arrays and concatenates per-head outputs.
"""
import sys

try:
    import concourse.bass as bass  # noqa: F401
except ImportError:
    sys.path.insert(0, "/opt/trn_rl_repo")
    import concourse.bass as bass  # noqa: F401

import numpy as np
import ml_dtypes
import concourse.tile as tile
from concourse import bacc, mybir
from concourse.bass_utils import run_bass_kernel_spmd

F32 = mybir.dt.float32
F32R = mybir.dt.float32r
F16 = mybir.dt.float16
BF16 = mybir.dt.bfloat16
AF = mybir.ActivationFunctionType
OP = mybir.AluOpType

T = 4096          # tokens (64*64)
C = 512           # channels
H = 8             # heads
CH = C // H       # 64 head dim
G = 32            # groupnorm groups
EPS = 1e-6
QKS = float(CH) ** -0.25   # scale applied to q and k each
NCORES = 8
TC = 512          # t-chunk width
NTC = T // TC     # 8 chunks
NSB = T // 128    # 32 s-blocks
NWARM = 28        # PE warmup matmuls
DEBUG_DUMP = False # add debug DRAM dumps of intermediates


def build_nc():
    nc = bacc.Bacc()
    xt = nc.declare_dram_parameter("xt", [C, T], BF16, isOutput=False)
    wqk = nc.declare_dram_parameter("wqk", [C, 128], BF16, isOutput=False)
    wv = nc.declare_dram_parameter("wv", [C, CH], BF16, isOutput=False)
    bqk = nc.declare_dram_parameter("bqk", [128, 1], F32, isOutput=False)
    bvT = nc.declare_dram_parameter("bvT", [CH, 1], F32, isOutput=False)
    gns = nc.declare_dram_parameter("gns", [128, 4], F32, isOutput=False)
    gnb = nc.declare_dram_parameter("gnb", [128, 4], F32, isOutput=False)
    ind = nc.declare_dram_parameter("ind", [128, 8], F32R, isOutput=False)
    indt = nc.declare_dram_parameter("indt", [8, 128], F32R, isOutput=False)
    outT = nc.declare_dram_parameter("outT", [T, CH], F32, isOutput=True)
    dbg = None
    if DEBUG_DUMP:
        dbg = {
            "dbg_vt": nc.declare_dram_parameter("dbg_vt", [128, NSB * 80], F16, isOutput=True),
            "dbg_qk1": nc.declare_dram_parameter("dbg_qk1", [128, T], F16, isOutput=True),
            "dbg_kq2": nc.declare_dram_parameter("dbg_kq2", [128, T], F16, isOutput=True),
        }

    with tile.TileContext(nc) as tc:
        _build_body(nc, tc, xt, wqk, wv, bqk, bvT, gns, gnb, ind, indt, outT,
                    dbg)
    nc.finalize()
    return nc


def _build_body(nc, tc, xt, wqk, wv, bqk, bvT, gns, gnb, ind, indt, outT,
                dbg=None):
    from contextlib import ExitStack
    ctx = ExitStack()
    with ctx:
        const = ctx.enter_context(tc.tile_pool(name="const", bufs=1))
        big = ctx.enter_context(tc.tile_pool(name="big", bufs=1))
        work = ctx.enter_context(tc.tile_pool(name="work", bufs=2))
        epi = ctx.enter_context(tc.tile_pool(name="epi", bufs=2))
        ppool = ctx.enter_context(tc.tile_pool(name="ppool", bufs=6))
        ps_pair = ctx.enter_context(tc.tile_pool(name="ps_pair", bufs=2, space="PSUM"))
        ps_o = ctx.enter_context(tc.tile_pool(name="ps_o", bufs=2, space="PSUM"))
        ps_sm = ps_o

        # ---------------- loads ----------------
        # weights first (small) so the PE warmup can start immediately
        wqk_raw, wv_raw = [], []
        for k in range(4):
            a = const.tile([128, 128], BF16, tag=f"wqkr{k}")
            nc.sync.dma_start(out=a, in_=wqk[128 * k:128 * (k + 1), :])
            wqk_raw.append(a)
            v = const.tile([128, CH], BF16, tag=f"wvr{k}")
            nc.sync.dma_start(out=v, in_=wv[128 * k:128 * (k + 1), :])
            wv_raw.append(v)
        # whole-tile x loads (8KB contiguous rows aggregate best); arrival
        # order 0,2,1,3 so the DVE (tiles 0,1,3) and ACT (tile 2) stats
        # pipelines both start as early as possible.
        xtk = [None] * 4
        for k in (0, 2, 1, 3):
            t = big.tile([128, T], BF16, tag=f"xt{k}")
            nc.sync.dma_start(out=t, in_=xt[128 * k:128 * (k + 1), :])
            xtk[k] = t
        bqk_t = const.tile([128, 1], F32, tag="bqk")
        nc.sync.dma_start(out=bqk_t, in_=bqk[:, :])
        bvT_t = const.tile([CH, 1], F32, tag="bvT")
        nc.sync.dma_start(out=bvT_t, in_=bvT[:, :])
        gns_t = const.tile([128, 4], F32, tag="gns")
        nc.sync.dma_start(out=gns_t, in_=gns[:, :])
        gnb_t = const.tile([128, 4], F32, tag="gnb")
        nc.sync.dma_start(out=gnb_t, in_=gnb[:, :])
        ind_t = const.tile([128, 8], F32R, tag="ind")
        nc.sync.dma_start(out=ind_t, in_=ind[:, :])
        indt_t = const.tile([8, 128], F32R, tag="indt")
        nc.sync.dma_start(out=indt_t, in_=indt[:, :])

        # f16 identity for PE transposes (V^T -> vt, epilogue -> token-major)
        ident = const.tile([128, 128], F16, tag="ident")
        from concourse.masks import make_identity
        make_identity(nc, ident)

        # PE warmup: dummy matmuls paced by the arriving xt quarters keep
        # the HAM activity monitor warm through the DMA+stats startup so
        # phase B starts at 2.4 GHz instead of 1.2 GHz. Results unused.
        warm_ps = ps_sm.tile([128, TC], F32, name="warm_ps", tag="psO")
        for w_i in range(NWARM):
            k = w_i % 4
            q = (w_i // 4) % 4
            nc.tensor.matmul(out=warm_ps, lhsT=wqk_raw[k],
                             rhs=xtk[k][:, bass.ts(q, 1024)][:, 0:512],
                             start=True, stop=True)

        # ---------------- phase A: groupnorm stats -> folded weights ------
        # DVE bn_stats handles k-tiles {0,1,3}; the (otherwise idle) ACT
        # engine handles tile 2 via accum_out running sums of x and x^2.
        RS = work.tile([128, 8], F32R, tag="RS")
        tmp1 = work.tile([128, 1], F32, tag="tmp1")
        scr = big.tile([128, T], BF16, tag="scr")
        sx = work.tile([128, 1], F32, tag="sx")
        sx2 = work.tile([128, 1], F32, tag="sx2")
        for k in range(4):
            if k == 2:
                nc.scalar.activation(out=scr, in_=xtk[k], func=AF.Identity,
                                     accum_out=sx)
                nc.scalar.activation(out=scr, in_=xtk[k], func=AF.Square,
                                     accum_out=sx2)
                nc.vector.tensor_scalar_mul(out=RS[:, 2 * k:2 * k + 1],
                                            in0=sx, scalar1=1.0 / T)
                nc.vector.tensor_scalar_mul(out=RS[:, 2 * k + 1:2 * k + 2],
                                            in0=sx2, scalar1=1.0 / T)
                continue
            st = work.tile([128, 8, 6], F32, tag="st")
            for sub in range(8):
                nc.vector.bn_stats(out=st[:, sub, :],
                                   in_=xtk[k][:, 512 * sub:512 * (sub + 1)])
            mv = work.tile([128, 2], F32, tag="mv")
            nc.vector.bn_aggr(out=mv, in_=st)
            # RS[:, 2k] = mean ; RS[:, 2k+1] = var + mean^2 = E[x^2]
            nc.vector.tensor_copy(out=RS[:, 2 * k:2 * k + 1], in_=mv[:, 0:1])
            nc.vector.tensor_mul(out=tmp1, in0=mv[:, 0:1], in1=mv[:, 0:1])
            nc.vector.tensor_add(out=RS[:, 2 * k + 1:2 * k + 2], in0=tmp1,
                                 in1=mv[:, 1:2])
        psG = ps_sm.tile([128, 8], F32, tag="psO")
        nc.tensor.matmul(out=psG[0:8, :], lhsT=ind_t, rhs=RS, start=True,
                         stop=True)
        # group stats (8 groups-per-tile x 4 tiles)
        gm = work.tile([8, 4], F32, tag="gm")
        nc.vector.tensor_scalar_mul(out=gm, in0=psG[0:8, 0:8:2], scalar1=1.0 / 16)
        ge2 = work.tile([8, 4], F32, tag="ge2")
        nc.vector.tensor_scalar_mul(out=ge2, in0=psG[0:8, 1:8:2], scalar1=1.0 / 16)
        gv = work.tile([8, 4], F32, tag="gv")
        t8 = work.tile([8, 4], F32, tag="t8")
        nc.vector.tensor_mul(out=t8, in0=gm, in1=gm)
        nc.vector.tensor_sub(out=gv, in0=ge2, in1=t8)
        # rstd = rsqrt(var+eps), one Newton step for accuracy
        gve = work.tile([8, 4], F32, tag="gve")
        nc.vector.tensor_scalar_add(out=gve, in0=gv, scalar1=EPS)
        sq0 = work.tile([8, 4], F32, tag="sq0")
        nc.scalar.activation(out=sq0, in_=gve, func=AF.Sqrt)
        r0 = work.tile([8, 4], F32, tag="r0")
        nc.vector.reciprocal(out=r0, in_=sq0)
        nc.vector.tensor_mul(out=t8, in0=r0, in1=r0)
        nc.vector.tensor_mul(out=t8, in0=t8, in1=gve)
        nc.vector.tensor_scalar(out=t8, in0=t8, scalar1=-0.5, scalar2=1.5,
                                op0=OP.mult, op1=OP.add)
        grstd = work.tile([8, 4], F32, tag="grstd")
        nc.vector.tensor_mul(out=grstd, in0=r0, in1=t8)
        # G[:, 2k] = gmean_k ; G[:, 2k+1] = grstd_k
        Gt = work.tile([8, 8], F32R, tag="Gt")
        nc.vector.tensor_copy(out=Gt[:, 0:8:2], in_=gm)
        nc.vector.tensor_copy(out=Gt[:, 1:8:2], in_=grstd)
        psB = ps_sm.tile([128, 8], F32, tag="psO")
        nc.tensor.matmul(out=psB, lhsT=indt_t, rhs=Gt, start=True, stop=True)
        # per-channel A = rstd*gn_scale ; B = gn_bias - mean*A
        Av = work.tile([128, 4], F32, tag="Av")
        nc.vector.tensor_mul(out=Av, in0=psB[:, 1:8:2], in1=gns_t)
        t128 = work.tile([128, 4], F32, tag="t128")
        nc.vector.tensor_mul(out=t128, in0=psB[:, 0:8:2], in1=Av)
        Bc = work.tile([128, 4], F32, tag="Bc")
        nc.vector.tensor_sub(out=Bc, in0=gnb_t, in1=t128)
        Br = work.tile([128, 4], F32, tag="Br")
        nc.vector.tensor_copy(out=Br, in_=Bc)
        Bsr = work.tile([128, 4], F32, tag="Bsr")
        nc.vector.tensor_scalar_mul(out=Bsr, in0=Bc, scalar1=QKS)
        # column-duplicated copies: fp32r matmuls need an even moving dim
        Bsr2 = work.tile([128, 8], BF16, tag="Bsr2")
        nc.vector.tensor_copy(out=Bsr2[:, 0:8:2], in_=Bsr)
        nc.vector.tensor_copy(out=Bsr2[:, 1:8:2], in_=Bsr)
        Br2 = work.tile([128, 8], BF16, tag="Br2")
        nc.vector.tensor_copy(out=Br2[:, 0:8:2], in_=Br)
        nc.vector.tensor_copy(out=Br2[:, 1:8:2], in_=Br)
        Aqk = work.tile([128, 4], F32, tag="Aqk")
        nc.vector.tensor_scalar_mul(out=Aqk, in0=Av, scalar1=QKS)
        # scaled weights, cast to bf16 to match the bf16 x
        wqk_s, wv_s = [], []
        for k in range(4):
            a = const.tile([128, 128], BF16, tag=f"wqks{k}")
            nc.vector.tensor_scalar_mul(out=a, in0=wqk_raw[k],
                                        scalar1=Aqk[:, k:k + 1])
            wqk_s.append(a)
            v = const.tile([128, CH], BF16, tag=f"wvs{k}")
            nc.vector.tensor_scalar_mul(out=v, in0=wv_raw[k],
                                        scalar1=Av[:, k:k + 1])
            wv_s.append(v)
        # folded biases: b'' = s*b_host + (s*B) @ W_raw   (qk)  /  b + B @ Wv
        psQKb = ps_sm.tile([128, 2], F32, tag="psO")
        for k in range(4):
            nc.tensor.matmul(out=psQKb, lhsT=wqk_raw[k],
                             rhs=Bsr2[:, 2 * k:2 * k + 2],
                             start=(k == 0), stop=(k == 3))
        bqk_dev = const.tile([128, 1], F32, tag="bqk_dev")
        nc.vector.tensor_add(out=bqk_dev, in0=psQKb[:, 0:1], in1=bqk_t)
        psBvT = ps_sm.tile([CH, 2], F32, tag="psO")
        for k in range(4):
            nc.tensor.matmul(out=psBvT, lhsT=wv_raw[k],
                             rhs=Br2[:, 2 * k:2 * k + 2],
                             start=(k == 0), stop=(k == 3))
        bv_dev = const.tile([CH, 1], F32, tag="bv_dev")
        nc.vector.tensor_add(out=bv_dev, in0=psBvT[:, 0:1], in1=bvT_t)

        # ---------------- phase B: projections ----------------
        qk1 = big.tile([128, T], F16, tag="qk1")   # [Q; K]
        kq2 = big.tile([128, T], F16, tag="kq2")   # [K; Q] via DMA copies

        for cc in range(NTC):
            sl = bass.ts(cc, TC)
            ps1 = ps_pair.tile([128, 1536], F32, tag="pp")
            for k in range(4):
                nc.tensor.matmul(out=ps1[:, 0:512], lhsT=wqk_s[k],
                                 rhs=xtk[k][:, sl], start=(k == 0),
                                 stop=(k == 3))
            # bias-add on ACT (idle during this phase), writes f16
            nc.scalar.activation(out=qk1[:, sl], in_=ps1[:, 0:512],
                                 func=AF.Identity, bias=bqk_dev, scale=1.0)
            # swapped layout via SBUF->SBUF DMA (partition-crossing copy)
            nc.sync.dma_start(out=kq2[0:64, sl], in_=qk1[64:128, sl])
            nc.sync.dma_start(out=kq2[64:128, sl], in_=qk1[0:64, sl])
        # V^T projection, then DMA-transpose (XBAR) into token-major vt
        # (with a ones column for the softmax denominator). The transposes
        # run on the idle DMA engines instead of the PE.
        vts = big.tile([CH, T], F16, tag="vts")
        # inner stride 80: XBAR-transpose output rows need 32-byte alignment
        vt = big.tile([128, NSB, 80], F16, tag="vt")
        nc.vector.memset(vt[:, :, CH:CH + 1], 1.0)

        def vts_block():
            # emitted inside chunk 0, after its first few QK^T pairs, so the
            # early softmax exps overlap the V projection on the ACT engine
            for vc in range(NTC):
                vsl = bass.ts(vc, TC)
                psVT = ps_sm.tile([CH, TC], F32, name=f"psVT_{vc}", tag="psO")
                for k in range(4):
                    nc.tensor.matmul(out=psVT, lhsT=wv_s[k],
                                     rhs=xtk[k][:, vsl],
                                     start=(k == 0), stop=(k == 3))
                nc.scalar.activation(out=vts[:, vsl], in_=psVT,
                                     func=AF.Identity, bias=bv_dev, scale=1.0)
                # one XBAR transpose per chunk: out[p,j,c] = vts[c, j*128+p]
                nc.sync.dma_start(out=vt[:, 4 * vc:4 * vc + 4, 0:CH],
                                  in_=vts[:, vsl], transpose=True)

        # ---------------- phase C: attention ----------------
        # S^T tiles grouped 3 s-blocks wide so each ACT exp covers 1536
        # columns (amortizes per-instruction overhead on the bottleneck
        # engine). mm1 runs as concurrent 64x128 row-tile pairs.
        NP = NSB // 2
        NT_C = 11  # first tile 2 blocks (early first exp), then 10 of 3

        def tile_of(blk):
            if blk < 2:
                return 0, blk
            return (blk - 2) // 3 + 1, (blk - 2) % 3

        def nblk_of(ti):
            return 2 if ti == 0 else 3

        def last_blk_of(ti):
            return 1 if ti == 0 else 3 * ti + 1

        def epi_finish(state):
            # psO holds [out^T; den] (65 x 512). Transpose to token-major so
            # the denominator spreads across partitions, then the reciprocal
            # and the normalizing multiply are cheap per-partition DVE ops.
            cc0, sAB0 = state
            # inner dim padded to CH+2 so each j-slice is 4-byte aligned in PSUM
            psT = ps_o.tile([128, 4, CH + 2], F16, name=f"psT_{cc0}", tag="psO")
            for j in range(4):
                nc.tensor.transpose(out=psT[:, j, 0:CH + 1],
                                    in_=sAB0[:, bass.ts(j, 128)],
                                    identity=ident[0:CH + 1, 0:CH + 1])
            rden = work.tile([128, 4], F32, name=f"rden_{cc0}", tag="rden")
            nc.vector.reciprocal(out=rden, in_=psT[:, :, CH])
            osb = work.tile([128, 4, CH], F32, name=f"osb_{cc0}", tag="osb")
            for j in range(4):
                nc.vector.tensor_scalar_mul(out=osb[:, j, :],
                                            in0=psT[:, j, 0:CH],
                                            scalar1=rden[:, j:j + 1])
                nc.sync.dma_start(
                    out=outT[TC * cc0 + 128 * j:TC * cc0 + 128 * (j + 1), :],
                    in_=osb[:, j, :])

        def mm1_pair(cc, pr, tiles):
            sl = bass.ts(cc, TC)
            for half, blk in ((0, 2 * pr), (1, 2 * pr + 1)):
                ti, col = tile_of(blk)
                dst = tiles[ti][1][:, 512 * col:512 * (col + 1)]
                if half == 0:
                    nc.tensor.matmul(out=dst,
                                     lhsT=kq2[0:64, bass.ts(blk, 128)],
                                     rhs=qk1[0:64, sl], start=True, stop=True,
                                     tile_position=(0, 0))
                else:
                    nc.tensor.matmul(out=dst,
                                     lhsT=qk1[64:128, bass.ts(blk, 128)],
                                     rhs=kq2[64:128, sl], start=True,
                                     stop=True, tile_position=(64, 0))

        def exp_tile(tiles, ti):
            nblk, pp = tiles[ti]
            w = 512 * nblk
            P = ppool.tile([128, 1536], F16, tag="P")
            nc.scalar.activation(out=P[:, 0:w], in_=pp[:, 0:w], func=AF.Exp)
            return P

        def mm2_tile(psO, ti, P, nblk, first, last):
            for c in range(nblk):
                blk = c if ti == 0 else 3 * ti - 1 + c
                nc.tensor.matmul(out=psO[0:CH + 1, :],
                                 lhsT=vt[:, blk, 0:CH + 1],
                                 rhs=P[:, 512 * c:512 * (c + 1)],
                                 start=(first and c == 0),
                                 stop=(last and c == nblk - 1))

        pending = None       # (cc, sAB) awaiting the epilogue
        pending_last = None  # final AV tile of the previous chunk
        for cc in range(NTC):
            # chunk 0: psO allocation deferred past the V block so the ps_o
            # PSUM ring stays strictly sequential-use
            psO = None
            if cc > 0:
                psO = ps_o.tile([128, TC], F32, name=f"psO_{cc}", tag="psO")
            mm2_upto = 0
            tiles = {}
            done_upto = -1
            flushed = []   # (ti, P, nblk)
            for pr in range(NP):
                for blk in (2 * pr, 2 * pr + 1):
                    ti = tile_of(blk)[0]
                    if ti not in tiles:
                        tiles[ti] = (nblk_of(ti), ps_pair.tile(
                            [128, 1536], F32, name=f"pp_{cc}_{ti}", tag="pp"))
                mm1_pair(cc, pr, tiles)
                if pending_last is not None:
                    # previous chunk's last AV matmul + PSUM pull, deferred
                    # past this chunk's first QK pair so the next exp is
                    # never blocked on them
                    pcc, ppsO, plti, plP, pln = pending_last
                    mm2_tile(ppsO, plti, plP, pln, first=False, last=True)
                    sAB = epi.tile([CH + 1, TC], F16, name=f"sAB_{pcc}",
                                   tag="sAB")
                    nc.vector.tensor_copy(out=sAB, in_=ppsO[0:CH + 1, :])
                    pending = (pcc, sAB)
                    pending_last = None
                if cc == 0 and pr == 5:
                    vts_block()
                    psO = ps_o.tile([128, TC], F32, name="psO_0", tag="psO")
                pair_done = 2 * pr + 1
                while True:
                    ti = done_upto + 1
                    if ti >= NT_C or ti not in tiles:
                        break
                    nblk = tiles[ti][0]
                    if last_blk_of(ti) > pair_done:
                        break
                    P = exp_tile(tiles, ti)
                    flushed.append((ti, P, nblk))
                    done_upto = ti
                    if pending is not None and ti == 2:
                        epi_finish(pending)
                        pending = None
                if psO is not None:
                    while mm2_upto < len(flushed) - 1:
                        pti, pP, pn = flushed[mm2_upto]
                        mm2_tile(psO, pti, pP, pn, first=(pti == 0),
                                 last=False)
                        mm2_upto += 1
            lti, lP, ln = flushed[-1]
            pending_last = (cc, psO, lti, lP, ln)
        pcc, ppsO, plti, plP, pln = pending_last
        mm2_tile(ppsO, plti, plP, pln, first=False, last=True)
        if dbg is not None:
            nc.sync.dma_start(out=dbg["dbg_vt"][:, :], in_=vt[:, :, :])
            nc.sync.dma_start(out=dbg["dbg_qk1"][:, :], in_=qk1[:, :])
            nc.sync.dma_start(out=dbg["dbg_kq2"][:, :], in_=kq2[:, :])
        sAB = epi.tile([CH + 1, TC], F16, name=f"sAB_{pcc}", tag="sAB")
        nc.vector.tensor_copy(out=sAB, in_=ppsO[0:CH + 1, :])
        epi_finish((pcc, sAB))


_NC_CACHE = None
TRACE = False          # set True (e.g. from test.py) to capture an NTFF profile
TMPDIR = None          # set to a path to keep NEFF/NTFF artifacts
LAST_RESULT = None     # BassKernelResults of the most recent kernel() call


def _get_nc():
    global _NC_CACHE
    if _NC_CACHE is None:
        _NC_CACHE = build_nc()
    return _NC_CACHE


def kernel(x, gn_scale, gn_bias, w_qkv, b_qkv):
    x = np.asarray(x, dtype=np.float32)
    gn_scale = np.asarray(gn_scale, dtype=np.float32)
    gn_bias = np.asarray(gn_bias, dtype=np.float32)
    w_qkv = np.asarray(w_qkv, dtype=np.float32)
    b_qkv = np.asarray(b_qkv, dtype=np.float32)

    spatial = x.shape[:-1]
    xt = np.ascontiguousarray(x.reshape(T, C).T).astype(ml_dtypes.bfloat16)
    w3 = w_qkv.reshape(C, C, 3)
    wq, wk, wv = w3[..., 0], w3[..., 1], w3[..., 2]          # (C, C) each
    b3 = b_qkv.reshape(C, 3)
    bq, bk, bvv = b3[:, 0], b3[:, 1], b3[:, 2]

    # indicator matrices: channel partition p -> group j = p // 16
    p_idx = np.arange(128)
    ind = (p_idx[:, None] // 16 == np.arange(8)[None, :]).astype(np.float32)
    indt = np.ascontiguousarray(ind.T)
    gns_dev = np.ascontiguousarray(gn_scale.reshape(4, 128).T)
    gnb_dev = np.ascontiguousarray(gn_bias.reshape(4, 128).T)

    in_maps = []
    for h in range(H):
        hsl = slice(CH * h, CH * (h + 1))
        wq_h, wk_h, wv_h = wq[:, hsl], wk[:, hsl], wv[:, hsl]
        in_maps.append({
            "xt": xt,
            "wqk": np.concatenate([wq_h, wk_h], axis=1).astype(ml_dtypes.bfloat16),
            "wv": np.ascontiguousarray(wv_h).astype(ml_dtypes.bfloat16),
            "bqk": (QKS * np.concatenate([bq[hsl], bk[hsl]])).reshape(128, 1),
            "bvT": bvv[hsl].reshape(CH, 1).copy(),
            "gns": gns_dev,
            "gnb": gnb_dev,
            "ind": ind,
            "indt": indt,
        })

    nc = _get_nc()
    res = run_bass_kernel_spmd(nc, in_maps, list(range(NCORES)), trace=TRACE,
                               tmpdir=TMPDIR)
    global LAST_RESULT
    LAST_RESULT = res
    out = np.empty((T, C), dtype=np.float32)
    for h in range(H):
        out[:, CH * h:CH * (h + 1)] = res.results[h]["outT"]
    return out.reshape(spatial + (C,))
